# revision 14
# baseline (speedup 1.0000x reference)
"""AdditiveAttention distributed Bass kernel for 8 TRN2 NeuronCores.

Data-parallel over batch: B=8 samples -> 1 per core. Weights replicated.

Per-core math (S=2048, D=1024, H=16, HD=64, sc=1/sqrt(HD)):
  q = X @ W_qv + b_qv ; v = q ; k = X @ W_k + b_k
  alphas = softmax_h((q @ Wq_s + bq_s) * sc)
  gq[d]  = sum_s alphas[s, h(d)] * q[s, d]          h(d) = d // 64
  betas  = softmax_h(((k*gq) @ Wk_s + bk_s) * sc)
  gk[d]  = gq[d] * sum_s betas[s, h(d)] * k[s, d]
  out    = q + (q*gk) @ W_r + b_r

v3 restructure (kept): never materialize q or k; everything is X-based until
one fused output GEMM:
  logits_q^T = Wsm^T X^T,  Wsm = sc*(W_qv Wq_s)  (host),  + c0q bias in exp
  gq: A = X^T alpha, graw^T = A^T W_qv + s b^T, gq = masked diag extract
  logits_b^T = Wfold^T X^T, Wfold = W_k (sc*diag(gq) Wk_s)
  gk analogous via W_k and A_k
  out = X @ Wbig + b_out,  Wbig = W_qv (I + diag(gk) W_r),  b_out = b_qv@M + b_r

v5: the measured DMA bus is ~335 GB/s aggregate (~160 per queue), which makes
the input stream the long pole; and LoadStationary (~100ns, hidden only under
wide moving operands) dominates any matmul with a skinny moving side.
  - X natural is NOT shipped: xnat is derived from X^T by PE transposes that
    fill the DMA-wait bubbles (saves 4MB = ~12us of stream).
  - identity/masks ship from the host inside the wsm/packA transfers: gpsimd
    runs nothing but DMA issues (its affine_selects were serialized behind
    the SWDGE queue and stalled the first softmax by 9us).
  - graw and Wfold run in the moving-bound orientation (stationary = the
    [128,16] A^T / t chunks, moving = the 512-wide weight) instead of 64
    stationary reloads each; results transposed back on PE.
  - softmax+A pipelined per 4-s-block group; softmax2 sh-outer; warm-up
    matmuls keep the PE p-state up during DMA waits; fold chases per-chunk
    diag(gk) scales; gpsimd carries no late output tiles (drain tail).

All matmuls bf16 with f32 PSUM. Output stored bf16, host upcasts.
"""

import math
import os
from contextlib import ExitStack

import numpy as np

B, S, D, H = 8, 2048, 1024, 16
HD = D // H
SCALE = 1.0 / math.sqrt(HD)
NCORES = 8
P = 128
NDB = D // P      # 8 d-blocks
NSB = S // P      # 16 s-blocks
NCC = D // P      # 8 contraction chunks
SH = 512
NSH = S // SH     # 4
NDH = D // SH     # 2
NG = 4            # s-block group size for softmax pipelining

_CACHE = {}


def _build():
    import concourse.bacc as bacc
    import concourse.tile as tile
    import concourse.mybir as mybir

    f32 = mybir.dt.float32
    bf16 = mybir.dt.bfloat16
    AF = mybir.ActivationFunctionType
    ALU = mybir.AluOpType
    AXX = mybir.AxisListType.X

    nc = bacc.Bacc("TRN2", target_bir_lowering=False, debug=False,
                   num_devices=NCORES)

    # bulk tensors HOST-PRE-PERMUTED into SBUF layout [128, N]
    XT = nc.dram_tensor("XTb", [P, NCC * S], bf16, kind="ExternalInput").ap()
    Wqv = nc.dram_tensor("Wqvb", [P, NCC * D], bf16, kind="ExternalInput").ap()
    WqvT = nc.dram_tensor("WqvTb", [P, NCC * D], bf16, kind="ExternalInput").ap()
    Wk = nc.dram_tensor("Wkb", [P, NCC * D], bf16, kind="ExternalInput").ap()
    WkT = nc.dram_tensor("WkTb", [P, NCC * D], bf16, kind="ExternalInput").ap()
    Wr = nc.dram_tensor("Wrb", [P, NCC * D], bf16, kind="ExternalInput").ap()
    # packW = Wsm | eye128   (id16 = eye[:16,:16])
    PackW = nc.dram_tensor("packW", [P, NDB * H + P], bf16,
                           kind="ExternalInput").ap()
    # packA = Wks | bqvpp | bkpp | mask3S | mask3K
    NPA = NDB * H + 2 * NDB + 2 * NDB * H
    PackA = nc.dram_tensor("packA", [P, NPA], bf16, kind="ExternalInput").ap()
    # packR = bqv_row | bk_row
    PackR = nc.dram_tensor("packR", [1, 2 * D], bf16, kind="ExternalInput").ap()
    c0q = nc.dram_tensor("c0qf", [H], f32, kind="ExternalInput").ap()
    bks_s = nc.dram_tensor("bks_sf", [H], f32, kind="ExternalInput").ap()
    br_f = nc.dram_tensor("br_f32", [D], f32, kind="ExternalInput").ap()
    OUT = nc.dram_tensor("out", [S, D], bf16, kind="ExternalOutput").ap()

    with tile.TileContext(nc) as tc, ExitStack() as ctx:
        sbp = ctx.enter_context(tc.tile_pool(name="sbp", bufs=1))
        psp = ctx.enter_context(tc.tile_pool(name="psp", bufs=1, space="PSUM"))

        def st(shape, dt_, tag, bufs=1):
            return sbp.tile(shape, dt_, tag=tag, bufs=bufs, name=tag)

        def pt_(shape, tag, bufs, dt_=f32):
            return psp.tile(shape, dt_, tag=tag, bufs=bufs, name=tag)

        # ---------- resident big tensors ----------
        xt = st([P, NCC * S], bf16, "xt")       # X^T, chunk cc at cols cc*S
        xnat = st([P, NSB * D], bf16, "xnat")   # X natural (PE-derived)
        wqv_all = st([P, NCC * D], bf16, "wqv_all")
        wqvT_all = st([P, NCC * D], bf16, "wqvT_all")
        wk_all = st([P, NCC * D], bf16, "wk_all")
        wkT_all = st([P, NCC * D], bf16, "wkT_all")
        wr_all = st([P, NCC * D], bf16, "wr_all")   # becomes M = I+diag(gk)Wr
        wbig = st([P, NCC * D], bf16, "wbig")

        # ---------- small persistent ----------
        packW_sb = st([P, NDB * H + P], bf16, "packW_sb")
        wsm_sb = packW_sb[:, :NDB * H]
        eye_bf = packW_sb[:, NDB * H:]
        id16 = packW_sb[:16, NDB * H:NDB * H + 16]
        packA_sb = st([P, NPA], bf16, "packA_sb")
        wks_sb = packA_sb[:, :NDB * H]
        bqvpp = packA_sb[:, NDB * H:NDB * H + NDB]
        bkpp = packA_sb[:, NDB * H + NDB:NDB * H + 2 * NDB]
        mask3S = packA_sb[:, NDB * H + 2 * NDB:2 * NDB * H + 2 * NDB]
        mask3K = packA_sb[:, 2 * NDB * H + 2 * NDB:]
        packR_sb = st([1, 2 * D], bf16, "packR_sb")
        bqv_row = packR_sb[:1, :D]
        bk_row = packR_sb[:1, D:]
        t_sb = st([P, NDB * H], bf16, "t_sb")
        wfold = st([P, NDB * H], bf16, "wfold")
        c0q_sb = st([16, 1], f32, "c0q_sb")
        bks_sb = st([16, 1], f32, "bks_sb")
        c0k_sb = st([16, 1], f32, "c0k_sb")
        br_row = st([1, D], f32, "br_row")
        bout_sb = st([1, D], bf16, "bout_sb")
        boutB = st([P, D], f32, "boutB")

        eE = st([16, S], bf16, "eE")
        z_nat = st([P, NSB], f32, "z_nat")
        rz_nat = st([P, NSB], f32, "rz_nat")
        alpha = st([P, NSB * H], bf16, "alpha")
        beta = st([P, NSB * H], bf16, "beta")
        at_sb = st([16, D], bf16, "at_sb")
        gts = st([16, D], bf16, "gts")          # graw^T / Wfold^T staging
        a_nat = st([P, NCC * H], bf16, "a_nat")
        s_col = st([16, 1], bf16, "s_col")
        s_row = st([1, 16], bf16, "s_row")
        ext_tmp = st([P, NDB * H], f32, "ext_tmp")
        gq_sb = st([P, NDB], f32, "gq_sb")      # = SCALE * gq
        gkd_sb = st([P, NDB], f32, "gkd_sb")    # = gkd / SCALE
        gk_sb = st([P, NDB], f32, "gk_sb")

        # ---------- small DMAs on scalar (idle until the first exps) -------
        nc.scalar.dma_start(packW_sb[:], PackW[:, :])
        nc.scalar.dma_start(c0q_sb[:], c0q.unsqueeze(1))
        nc.scalar.dma_start(bks_sb[:], bks_s.unsqueeze(1))
        nc.scalar.dma_start(packR_sb[:], PackR[:, :])
        nc.scalar.dma_start(packA_sb[:], PackA[:, :])
        nc.scalar.dma_start(br_row[:], br_f.unsqueeze(0))

        # ---------- bulk DMA: stripe BOTH queues, global first-need order ---
        def xt_cc(cc, eng):
            eng.dma_start(xt[:, cc * S:(cc + 1) * S],
                          XT[:, cc * S:(cc + 1) * S])

        def w_quarter(dst, src_, qi, eng):
            eng.dma_start(dst[:, 2 * qi * D:(2 * qi + 2) * D],
                          src_[:, 2 * qi * D:(2 * qi + 2) * D])

        for cc in range(NCC):
            xt_cc(cc, [nc.gpsimd, nc.sync][cc % 2])
        for src_, dst in ((Wqv, wqv_all), (WkT, wkT_all), (Wk, wk_all),
                          (Wr, wr_all), (WqvT, wqvT_all)):
            for qi in range(4):
                w_quarter(dst, src_, qi, [nc.gpsimd, nc.sync][qi % 2])

        # ---------- tiny constants on vector ----------
        ones_col = st([P, 1], bf16, "ones_col")
        nc.vector.memset(ones_col[:], 1.0)
        ones_row = st([1, P], bf16, "ones_row")
        nc.vector.memset(ones_row[:], 1.0)

        # ---------- PE warm-up: junk matmuls on packW while DMA streams ----
        def warm(n):
            for _ in range(n):
                wt = pt_([P, SH], "big", 2)
                nc.tensor.matmul(wt[:, :P], packW_sb[:, :P], packW_sb[:, :P],
                                 start=True, stop=True)

        # ---------- xnat derivation: xt chunk cc -> transposed s-blocks ----
        # quad0 (cc 0..3): cc-major as the chunks land; xp tile holds 4
        # s-blocks of one cc, evacuated with a strided copy.
        # quad1 (cc 4..7): sb-major so A(sb) unblocks in order; contiguous.
        def xq_tr_q0(cc, g):
            xp = pt_([P, 4 * P], "sc", 2, dt_=bf16)
            for t in range(4):
                sb = 4 * g + t
                nc.tensor.transpose(xp[:, t * P:(t + 1) * P],
                                    xt[:, cc * S + sb * P: cc * S + (sb + 1) * P],
                                    eye_bf)
            dst = xnat[:].rearrange("p (sb c) -> p sb c", sb=NSB)[
                :, 4 * g:4 * g + 4, cc * P:(cc + 1) * P]
            nc.vector.tensor_copy(
                dst, xp[:].rearrange("p (sb c) -> p sb c", sb=4))

        def xq_tr_q1(sb, eng):
            xp = pt_([P, 4 * P], "sc", 2, dt_=bf16)
            for t in range(4):
                cc = 4 + t
                nc.tensor.transpose(xp[:, t * P:(t + 1) * P],
                                    xt[:, cc * S + sb * P: cc * S + (sb + 1) * P],
                                    eye_bf)
            dst = xnat[:, sb * D + 4 * P: sb * D + 8 * P]
            (eng.tensor_copy if eng is nc.vector else eng.copy)(dst, xp[:])

        # ---------- fused softmax -> weights -> A -> graw -> extract -------
        def softmax_ws(w16, bias_ap, wout, wall, b_row_ap, mask3, g_out,
                       sh_outer):
            lgs = [pt_([16, SH], "lg", 4) for _ in range(NSH)]
            if sh_outer:
                for sh in range(NSH):
                    for cb in range(NCC):
                        nc.tensor.matmul(
                            lgs[sh][:], w16[:, cb * H:(cb + 1) * H],
                            xt[:, cb * S + sh * SH: cb * S + sh * SH + SH],
                            start=(cb == 0), stop=(cb == NCC - 1))
                    nc.scalar.activation(eE[:, sh * SH:(sh + 1) * SH],
                                         lgs[sh][:], AF.Exp,
                                         bias=bias_ap, scale=1.0)
            else:
                # cc-outer: consume xt chunks as the DMA lands them; the
                # first-quad xnat transposes ride along with their chunk
                for cb in range(NCC):
                    for sh in range(NSH):
                        nc.tensor.matmul(
                            lgs[sh][:], w16[:, cb * H:(cb + 1) * H],
                            xt[:, cb * S + sh * SH: cb * S + sh * SH + SH],
                            start=(cb == 0), stop=(cb == NCC - 1))
                    if cb < 4:
                        for g in range(4):
                            xq_tr_q0(cb, g)
                    else:
                        warm(1)
                for sh in range(NSH):
                    nc.scalar.activation(eE[:, sh * SH:(sh + 1) * SH],
                                         lgs[sh][:], AF.Exp,
                                         bias=bias_ap, scale=1.0)

            a0 = pt_([16, SH], "lg", 4)
            a1 = pt_([16, SH], "lg", 4)
            sps_t = pt_([P, SH], "big", 2)
            sps = sps_t[:16, :1]

            def a_mms(sb):
                lhs = wout[:, sb * H:(sb + 1) * H]
                nc.tensor.matmul(a0[:], lhs,
                                 xnat[:, sb * D: sb * D + SH],
                                 start=(sb == 0), stop=(sb == NSB - 1))
                nc.tensor.matmul(a1[:], lhs,
                                 xnat[:, sb * D + SH: sb * D + 2 * SH],
                                 start=(sb == 0), stop=(sb == NSB - 1))

            ngrp = NSB // NG
            first = not sh_outer
            for g in range(ngrp):
                eT_ps = pt_([P, 4 * P], "sc", 2, dt_=bf16)
                for t in range(NG):
                    sb = NG * g + t
                    nc.tensor.transpose(eT_ps[:, t * H:(t + 1) * H],
                                        eE[:, sb * P:(sb + 1) * P], id16)
                sl = slice(NG * g * H, (NG * g + NG) * H)
                gz = slice(NG * g, NG * g + NG)
                nc.vector.reduce_sum(
                    z_nat[:, gz].unsqueeze(2),
                    eT_ps[:, :NG * H].rearrange("p (sb h) -> p sb h", sb=NG),
                    axis=AXX)
                nc.vector.reciprocal(rz_nat[:, gz], z_nat[:, gz])
                nc.vector.tensor_tensor(
                    wout[:, sl].rearrange("p (sb h) -> p sb h", sb=NG),
                    eT_ps[:, :NG * H].rearrange("p (sb h) -> p sb h", sb=NG),
                    rz_nat[:, gz].unsqueeze(2).broadcast_to([P, NG, H]),
                    ALU.mult)
                # second-quad xnat transposes + previous group's A matmuls
                # fill PE while this group's exp/alpha chain completes
                if first:
                    for sb in range(NG * g, NG * g + NG):
                        xq_tr_q1(sb, nc.scalar if sb % 2 == 1 and sb < 8
                                 else nc.vector)
                if g > 0:
                    for sb in range(NG * (g - 1), NG * g):
                        a_mms(sb)
            for sb in range(NG * (ngrp - 1), NSB):
                a_mms(sb)
            for sb in range(NSB):
                nc.tensor.matmul(sps, wout[:, sb * H:(sb + 1) * H],
                                 ones_col[:],
                                 start=(sb == 0), stop=(sb == NSB - 1))

            # ---- A^T -> natural ----
            nc.scalar.copy(at_sb[:, :SH], a0[:])
            nc.vector.tensor_copy(at_sb[:, SH:], a1[:])
            nc.vector.tensor_copy(s_col[:], sps)
            trA = pt_([P, 4 * P], "sc", 2, dt_=bf16)
            for cc in range(NCC):
                nc.tensor.transpose(trA[:, cc * H:(cc + 1) * H],
                                    at_sb[:, cc * P:(cc + 1) * P], id16)
            nc.tensor.transpose(trA[:1, NSB * H:NSB * H + 16], s_col[:], id16)
            nc.vector.tensor_copy(a_nat[:], trA[:, :NCC * H])
            nc.vector.tensor_copy(s_row[:], trA[:1, NSB * H:NSB * H + 16])

            # ---- graw^T = A^T-stationary x W-moving (+ s b^T), then back ---
            gA = pt_([16, SH], "lg", 4)
            gB = pt_([16, SH], "lg", 4)
            for dd in range(NCC):
                nc.tensor.matmul(gA[:], a_nat[:, dd * H:(dd + 1) * H],
                                 wall[:, dd * D: dd * D + SH],
                                 start=(dd == 0), stop=False)
                nc.tensor.matmul(gB[:], a_nat[:, dd * H:(dd + 1) * H],
                                 wall[:, dd * D + SH: (dd + 1) * D],
                                 start=(dd == 0), stop=False)
            nc.tensor.matmul(gA[:], s_row[:1, :], b_row_ap[:1, :SH],
                             start=False, stop=True)
            nc.tensor.matmul(gB[:], s_row[:1, :], b_row_ap[:1, SH:],
                             start=False, stop=True)
            nc.scalar.copy(gts[:, :SH], gA[:])
            nc.vector.tensor_copy(gts[:, SH:], gB[:])
            grT = pt_([P, 4 * P], "sc", 2, dt_=bf16)
            for jb in range(NDB):
                nc.tensor.transpose(grT[:, jb * H:(jb + 1) * H],
                                    gts[:, jb * P:(jb + 1) * P], id16)
            nc.vector.tensor_tensor(ext_tmp[:], grT[:, :NDB * H], mask3[:],
                                    ALU.mult)
            nc.vector.reduce_sum(
                g_out[:].unsqueeze(2),
                ext_tmp[:].rearrange("p (j h) -> p j h", j=NDB),
                axis=AXX)

        # ---------- phase 1: alphas -> gq ----------
        warm(4)
        softmax_ws(wsm_sb, c0q_sb[:, :1], alpha, wqv_all, bqv_row, mask3S,
                   gq_sb, sh_outer=False)

        # ---------- phase 2: t = sc*diag(gq)*Wk_s ; Wfold ; betas -> gkd ----
        nc.vector.tensor_tensor(
            t_sb[:].rearrange("p (j h) -> p j h", j=NDB),
            wks_sb[:].rearrange("p (j h) -> p j h", j=NDB),
            gq_sb[:].unsqueeze(2).broadcast_to([P, NDB, H]),
            ALU.mult)
        c0k_t = pt_([P, SH], "big", 2)
        c0k_ps = c0k_t[:16, :1]
        for j in range(NDB):
            nc.tensor.matmul(c0k_ps, t_sb[:, j * H:(j + 1) * H],
                             bkpp[:, j:j + 1],
                             start=(j == 0), stop=(j == NDB - 1))
        nc.vector.tensor_tensor(c0k_sb[:], c0k_ps, bks_sb[:], ALU.add)
        # Wfold^T = t-stationary x WkT-moving (dd-progressive), then back
        wfA = pt_([16, SH], "lg", 4)
        wfB = pt_([16, SH], "lg", 4)
        for dd in range(NCC):
            nc.tensor.matmul(wfA[:], t_sb[:, dd * H:(dd + 1) * H],
                             wkT_all[:, dd * D: dd * D + SH],
                             start=(dd == 0), stop=(dd == NCC - 1))
            nc.tensor.matmul(wfB[:], t_sb[:, dd * H:(dd + 1) * H],
                             wkT_all[:, dd * D + SH: (dd + 1) * D],
                             start=(dd == 0), stop=(dd == NCC - 1))
        nc.scalar.copy(gts[:, :SH], wfA[:])
        nc.vector.tensor_copy(gts[:, SH:], wfB[:])
        wf_t = pt_([P, 4 * P], "sc", 2, dt_=bf16)
        for jb in range(NDB):
            nc.tensor.transpose(wf_t[:, jb * H:(jb + 1) * H],
                                gts[:, jb * P:(jb + 1) * P], id16)
        nc.vector.tensor_copy(wfold[:], wf_t[:, :NDB * H])
        softmax_ws(wfold, c0k_sb[:, :1], beta, wk_all, bk_row, mask3K,
                   gkd_sb, sh_outer=True)

        # ---------- phase 3: gk ; M = I + diag(gk) Wr ----------
        nc.vector.tensor_mul(gk_sb[:], gq_sb[:], gkd_sb[:])
        for cc in range(NCC):
            sl = slice(cc * D, (cc + 1) * D)
            if cc % 2 == 0:
                nc.vector.tensor_scalar(wr_all[:, sl], wr_all[:, sl],
                                        gk_sb[:, cc:cc + 1], None, ALU.mult)
            else:
                nc.scalar.activation(wr_all[:, sl], wr_all[:, sl], AF.Copy,
                                     bias=0.0, scale=gk_sb[:, cc:cc + 1])
            nc.vector.tensor_add(
                wr_all[:, cc * D + cc * P: cc * D + (cc + 1) * P],
                wr_all[:, cc * D + cc * P: cc * D + (cc + 1) * P], eye_bf)
        # ---------- phase 4: Wbig = W_qv @ M  (chases the per-chunk scales) -
        for cb in range(NCC):
            for eh in range(NDH):
                ps = pt_([P, SH], "big", 2)
                for dd in range(NCC):
                    nc.tensor.matmul(
                        ps[:], wqvT_all[:, dd * D + cb * P: dd * D + cb * P + P],
                        wr_all[:, dd * D + eh * SH: dd * D + (eh + 1) * SH],
                        start=(dd == 0), stop=(dd == NCC - 1))
                nc.scalar.copy(wbig[:, cb * D + eh * SH: cb * D + (eh + 1) * SH],
                               ps[:])

        # b_out = b_qv @ M + b_r
        for eh in range(NDH):
            bo = pt_([16, SH], "lg", 4)
            for j in range(NDB):
                nc.tensor.matmul(bo[:1, :], bqvpp[:, j:j + 1],
                                 wr_all[:, j * D + eh * SH: j * D + (eh + 1) * SH],
                                 start=(j == 0), stop=(j == NDB - 1))
            nc.vector.tensor_tensor(bout_sb[:1, eh * SH:(eh + 1) * SH],
                                    bo[:1, :], br_row[:1, eh * SH:(eh + 1) * SH],
                                    ALU.add)
        for eh in range(NDH):
            bb = pt_([P, SH], "big", 2)
            nc.tensor.matmul(bb[:], ones_row[:1, :],
                             bout_sb[:1, eh * SH:(eh + 1) * SH],
                             start=True, stop=True)
            nc.vector.tensor_copy(boutB[:, eh * SH:(eh + 1) * SH], bb[:])

        # ---------- phase 5: out = X @ Wbig + b_out ----------
        for sb in range(NSB):
            for eh in range(NDH):
                ps = pt_([P, SH], "big", 2)
                for cc in range(NCC):
                    nc.tensor.matmul(
                        ps[:], xt[:, cc * S + sb * P: cc * S + sb * P + P],
                        wbig[:, cc * D + eh * SH: cc * D + (eh + 1) * SH],
                        start=(cc == 0), stop=(cc == NCC - 1))
                if sb == NSB - 1:
                    hw = SH // 2
                    for q in range(2):
                        obq = st([P, hw], bf16, "obq", bufs=4)
                        nc.vector.tensor_tensor(
                            obq[:], ps[:, q * hw:(q + 1) * hw],
                            boutB[:, eh * SH + q * hw: eh * SH + (q + 1) * hw],
                            ALU.add)
                        nc.sync.dma_start(
                            OUT[sb * P:(sb + 1) * P,
                                eh * SH + q * hw: eh * SH + (q + 1) * hw],
                            obq[:])
                else:
                    ob = st([P, SH], bf16, "ob", bufs=4)
                    nc.vector.tensor_tensor(
                        ob[:], ps[:], boutB[:, eh * SH:(eh + 1) * SH], ALU.add)
                    eng = nc.gpsimd if sb < 12 and (sb * NDH + eh) % 2 == 0 \
                        else nc.sync
                    eng.dma_start(
                        OUT[sb * P:(sb + 1) * P, eh * SH:(eh + 1) * SH],
                        ob[:])

    nc.compile()
    return nc


def _get_nc():
    if "nc" not in _CACHE:
        _CACHE["nc"] = _build()
    return _CACHE["nc"]


def _prep_inputs(inputs):
    import ml_dtypes
    bf = ml_dtypes.bfloat16

    def f(k):
        return np.ascontiguousarray(np.asarray(inputs[k], dtype=np.float32))

    def c(a):
        return np.ascontiguousarray(np.asarray(a, dtype=np.float32).astype(bf))

    W_qv, W_k, W_r = f("W_qv"), f("W_k"), f("W_r")
    Wq_s, Wk_s = f("Wq_s"), f("Wk_s")
    b_qv, b_k, b_r = f("b_qv"), f("b_k"), f("b_r")
    bq_s, bk_s = f("bq_s"), f("bk_s")

    def perm(w):
        # [C*128, N] -> [128, C*N]: row-block cc goes to columns cc*N
        cb = w.shape[0] // P
        return w.reshape(cb, P, w.shape[1]).transpose(1, 0, 2).reshape(P, -1)

    # mask3[p, j*H + h] = v iff h == 2j + p//64
    pj = np.arange(P)[:, None] // 64 + 2 * np.arange(NDB)[None, :]  # [P, j]
    m3 = (pj[:, :, None] == np.arange(H)[None, None, :])            # [P, j, H]
    mask3S = (m3 * SCALE).reshape(P, NDB * H)
    mask3K = (m3 / SCALE).reshape(P, NDB * H)

    packW = np.concatenate([perm(SCALE * (W_qv @ Wq_s)), np.eye(P)], axis=1)
    packA = np.concatenate(
        [perm(Wk_s), b_qv.reshape(NDB, P).T, b_k.reshape(NDB, P).T,
         mask3S, mask3K], axis=1)
    packR = np.concatenate([b_qv, b_k]).reshape(1, 2 * D)

    common = {
        "Wqvb": c(perm(W_qv)), "WqvTb": c(perm(W_qv.T)),
        "Wkb": c(perm(W_k)), "WkTb": c(perm(W_k.T)),
        "Wrb": c(perm(W_r)),
        "packW": c(packW),
        "packA": c(packA),
        "packR": c(packR),
        "c0qf": np.ascontiguousarray(SCALE * (b_qv @ Wq_s + bq_s)),
        "bks_sf": np.ascontiguousarray(SCALE * bk_s),
        "br_f32": b_r,
    }
    in_maps = []
    for b in range(NCORES):
        m = dict(common)
        xb = np.asarray(inputs["X"][b], dtype=np.float32)
        m["XTb"] = c(perm(xb.T))
        in_maps.append(m)
    return in_maps


def run(inputs, trace=False):
    from concourse.bass_utils import run_bass_kernel_spmd

    nc = _get_nc()
    in_maps = _prep_inputs(inputs)
    res = run_bass_kernel_spmd(nc, in_maps, core_ids=list(range(NCORES)),
                               trace=trace)
    _CACHE["last_results"] = res
    out = np.stack([np.asarray(res.results[b]["out"], dtype=np.float32)
                    for b in range(NCORES)], axis=0)
    return out


def kernel(**inputs):
    trace = os.environ.get("KTRACE", "0") == "1"
    return run(inputs, trace=trace)


# revision 15
# speedup vs baseline: 1.0296x; 1.0296x over previous
"""AdditiveAttention distributed Bass kernel for 8 TRN2 NeuronCores.

Data-parallel over batch: B=8 samples -> 1 per core. Weights replicated.

Per-core math (S=2048, D=1024, H=16, HD=64, sc=1/sqrt(HD)):
  q = X @ W_qv + b_qv ; v = q ; k = X @ W_k + b_k
  alphas = softmax_h((q @ Wq_s + bq_s) * sc)
  gq[d]  = sum_s alphas[s, h(d)] * q[s, d]          h(d) = d // 64
  betas  = softmax_h(((k*gq) @ Wk_s + bk_s) * sc)
  gk[d]  = gq[d] * sum_s betas[s, h(d)] * k[s, d]
  out    = q + (q*gk) @ W_r + b_r

v3 restructure (kept): never materialize q or k; everything is X-based until
one fused output GEMM:
  logits_q^T = Wsm^T X^T,  Wsm = sc*(W_qv Wq_s)  (host),  + c0q bias in exp
  gq: A = X^T alpha, graw^T = A^T W_qv + s b^T, gq = masked diag extract
  logits_b^T = Wfold^T X^T, Wfold = W_k (sc*diag(gq) Wk_s)
  gk analogous via W_k and A_k
  out = X @ Wbig + b_out,  Wbig = W_qv (I + diag(gk) W_r),  b_out = b_qv@M + b_r

v5: the measured DMA bus is ~335 GB/s aggregate (~160 per queue), which makes
the input stream the long pole; and LoadStationary (~100ns, hidden only under
wide moving operands) dominates any matmul with a skinny moving side.
  - X natural is NOT shipped: xnat is derived from X^T by PE transposes that
    fill the DMA-wait bubbles (saves 4MB = ~12us of stream).
  - identity/masks ship from the host inside the wsm/packA transfers: gpsimd
    runs nothing but DMA issues (its affine_selects were serialized behind
    the SWDGE queue and stalled the first softmax by 9us).
  - graw and Wfold run in the moving-bound orientation (stationary = the
    [128,16] A^T / t chunks, moving = the 512-wide weight) instead of 64
    stationary reloads each; results transposed back on PE.
  - softmax+A pipelined per 4-s-block group; softmax2 sh-outer; warm-up
    matmuls keep the PE p-state up during DMA waits; fold chases per-chunk
    diag(gk) scales; gpsimd carries no late output tiles (drain tail).

All matmuls bf16 with f32 PSUM. Output stored bf16, host upcasts.
"""

import math
import os
from contextlib import ExitStack

import numpy as np

B, S, D, H = 8, 2048, 1024, 16
HD = D // H
SCALE = 1.0 / math.sqrt(HD)
NCORES = 8
P = 128
NDB = D // P      # 8 d-blocks
NSB = S // P      # 16 s-blocks
NCC = D // P      # 8 contraction chunks
SH = 512
NSH = S // SH     # 4
NDH = D // SH     # 2
NG = 4            # s-block group size for softmax pipelining

_CACHE = {}


def _build():
    import concourse.bacc as bacc
    import concourse.tile as tile
    import concourse.mybir as mybir

    f32 = mybir.dt.float32
    bf16 = mybir.dt.bfloat16
    AF = mybir.ActivationFunctionType
    ALU = mybir.AluOpType
    AXX = mybir.AxisListType.X

    nc = bacc.Bacc("TRN2", target_bir_lowering=False, debug=False,
                   num_devices=NCORES)

    # bulk tensors HOST-PRE-PERMUTED into SBUF layout [128, N]
    XT = nc.dram_tensor("XTb", [P, NCC * S], bf16, kind="ExternalInput").ap()
    Wqv = nc.dram_tensor("Wqvb", [P, NCC * D], bf16, kind="ExternalInput").ap()
    WqvT = nc.dram_tensor("WqvTb", [P, NCC * D], bf16, kind="ExternalInput").ap()
    Wk = nc.dram_tensor("Wkb", [P, NCC * D], bf16, kind="ExternalInput").ap()
    WkT = nc.dram_tensor("WkTb", [P, NCC * D], bf16, kind="ExternalInput").ap()
    Wr = nc.dram_tensor("Wrb", [P, NCC * D], bf16, kind="ExternalInput").ap()
    # packW = Wsm | eye128   (id16 = eye[:16,:16])
    PackW = nc.dram_tensor("packW", [P, NDB * H + P], bf16,
                           kind="ExternalInput").ap()
    # packA = Wks | bqvpp | bkpp | mask3S | mask3K
    NPA = NDB * H + 2 * NDB + 2 * NDB * H
    PackA = nc.dram_tensor("packA", [P, NPA], bf16, kind="ExternalInput").ap()
    # packR = bqv_row | bk_row
    PackR = nc.dram_tensor("packR", [1, 2 * D], bf16, kind="ExternalInput").ap()
    c0q = nc.dram_tensor("c0qf", [H], f32, kind="ExternalInput").ap()
    bks_s = nc.dram_tensor("bks_sf", [H], f32, kind="ExternalInput").ap()
    br_f = nc.dram_tensor("br_f32", [D], f32, kind="ExternalInput").ap()
    OUT = nc.dram_tensor("out", [S, D], bf16, kind="ExternalOutput").ap()

    with tile.TileContext(nc) as tc, ExitStack() as ctx:
        sbp = ctx.enter_context(tc.tile_pool(name="sbp", bufs=1))
        psp = ctx.enter_context(tc.tile_pool(name="psp", bufs=1, space="PSUM"))

        def st(shape, dt_, tag, bufs=1):
            return sbp.tile(shape, dt_, tag=tag, bufs=bufs, name=tag)

        def pt_(shape, tag, bufs, dt_=f32):
            return psp.tile(shape, dt_, tag=tag, bufs=bufs, name=tag)

        # ---------- resident big tensors ----------
        xt = st([P, NCC * S], bf16, "xt")       # X^T, chunk cc at cols cc*S
        xnat = st([P, NSB * D], bf16, "xnat")   # X natural (PE-derived)
        wqv_all = st([P, NCC * D], bf16, "wqv_all")
        wqvT_all = st([P, NCC * D], bf16, "wqvT_all")
        wk_all = st([P, NCC * D], bf16, "wk_all")
        wkT_all = st([P, NCC * D], bf16, "wkT_all")
        wr_all = st([P, NCC * D], bf16, "wr_all")   # becomes M = I+diag(gk)Wr
        wbig = st([P, NCC * D], bf16, "wbig")

        # ---------- small persistent ----------
        packW_sb = st([P, NDB * H + P], bf16, "packW_sb")
        wsm_sb = packW_sb[:, :NDB * H]
        eye_bf = packW_sb[:, NDB * H:]
        id16 = packW_sb[:16, NDB * H:NDB * H + 16]
        packA_sb = st([P, NPA], bf16, "packA_sb")
        wks_sb = packA_sb[:, :NDB * H]
        bqvpp = packA_sb[:, NDB * H:NDB * H + NDB]
        bkpp = packA_sb[:, NDB * H + NDB:NDB * H + 2 * NDB]
        mask3S = packA_sb[:, NDB * H + 2 * NDB:2 * NDB * H + 2 * NDB]
        mask3K = packA_sb[:, 2 * NDB * H + 2 * NDB:]
        packR_sb = st([1, 2 * D], bf16, "packR_sb")
        bqv_row = packR_sb[:1, :D]
        bk_row = packR_sb[:1, D:]
        t_sb = st([P, NDB * H], bf16, "t_sb")
        wfold = st([P, NDB * H], bf16, "wfold")
        c0q_sb = st([16, 1], f32, "c0q_sb")
        bks_sb = st([16, 1], f32, "bks_sb")
        c0k_sb = st([16, 1], f32, "c0k_sb")
        br_row = st([1, D], f32, "br_row")
        bout_sb = st([1, D], bf16, "bout_sb")
        boutB = st([P, D], f32, "boutB")

        eE = st([16, S], bf16, "eE")
        z_nat = st([P, NSB], f32, "z_nat")
        rz_nat = st([P, NSB], f32, "rz_nat")
        alpha = st([P, NSB * H], bf16, "alpha")
        beta = st([P, NSB * H], bf16, "beta")
        at_sb = st([16, D], bf16, "at_sb")
        gts = st([16, D], bf16, "gts")          # graw^T / Wfold^T staging
        a_nat = st([P, NCC * H], bf16, "a_nat")
        s_col = st([16, 1], bf16, "s_col")
        s_row = st([1, 16], bf16, "s_row")
        ext_tmp = st([P, NDB * H], f32, "ext_tmp")
        gq_sb = st([P, NDB], f32, "gq_sb")      # = SCALE * gq
        gkd_sb = st([P, NDB], f32, "gkd_sb")    # = gkd / SCALE
        gk_sb = st([P, NDB], f32, "gk_sb")

        # ---------- small DMAs on scalar (idle until the first exps) -------
        nc.scalar.dma_start(packW_sb[:], PackW[:, :])
        nc.scalar.dma_start(c0q_sb[:], c0q.unsqueeze(1))
        nc.scalar.dma_start(bks_sb[:], bks_s.unsqueeze(1))
        nc.scalar.dma_start(packR_sb[:], PackR[:, :])
        nc.scalar.dma_start(packA_sb[:], PackA[:, :])
        nc.scalar.dma_start(br_row[:], br_f.unsqueeze(0))

        # ---------- bulk DMA: stripe BOTH queues, global first-need order ---
        def xt_cc(cc, eng):
            eng.dma_start(xt[:, cc * S:(cc + 1) * S],
                          XT[:, cc * S:(cc + 1) * S])

        def w_quarter(dst, src_, qi, eng):
            eng.dma_start(dst[:, 2 * qi * D:(2 * qi + 2) * D],
                          src_[:, 2 * qi * D:(2 * qi + 2) * D])

        # sync's HWDGE ring moves ~1.35x gpsimd's SWDGE queue: give sync ~60%
        # of the bytes so both finish together
        for cc in range(NCC):
            xt_cc(cc, nc.gpsimd if cc in (3, 5, 7) else nc.sync)
        wq_gp = {("Wqvb", 1), ("Wqvb", 2), ("WkTb", 0), ("WkTb", 2),
                 ("Wkb", 1), ("Wrb", 0), ("Wrb", 2), ("WqvTb", 1)}
        for src_, dst, nm in ((Wqv, wqv_all, "Wqvb"), (WkT, wkT_all, "WkTb"),
                              (Wk, wk_all, "Wkb"), (Wr, wr_all, "Wrb"),
                              (WqvT, wqvT_all, "WqvTb")):
            for qi in range(4):
                w_quarter(dst, src_, qi,
                          nc.gpsimd if (nm, qi) in wq_gp else nc.sync)

        # ---------- tiny constants on vector ----------
        ones_col = st([P, 1], bf16, "ones_col")
        nc.vector.memset(ones_col[:], 1.0)
        ones_row = st([1, P], bf16, "ones_row")
        nc.vector.memset(ones_row[:], 1.0)

        # ---------- PE warm-up: junk matmuls on packW while DMA streams ----
        def warm(n):
            for _ in range(n):
                wt = pt_([P, SH], "big", 2)
                nc.tensor.matmul(wt[:, :P], packW_sb[:, :P], packW_sb[:, :P],
                                 start=True, stop=True)

        # ---------- xnat derivation: xt chunk cc -> transposed s-blocks ----
        # quad0 (cc 0..3): cc-major as the chunks land; xp tile holds 4
        # s-blocks of one cc, evacuated with a strided copy.
        # quad1 (cc 4..7): sb-major so A(sb) unblocks in order; contiguous.
        def xq_tr_q0(cc, g):
            xp = pt_([P, 4 * P], "sc", 2, dt_=bf16)
            for t in range(4):
                sb = 4 * g + t
                nc.tensor.transpose(xp[:, t * P:(t + 1) * P],
                                    xt[:, cc * S + sb * P: cc * S + (sb + 1) * P],
                                    eye_bf)
            dst = xnat[:].rearrange("p (sb c) -> p sb c", sb=NSB)[
                :, 4 * g:4 * g + 4, cc * P:(cc + 1) * P]
            nc.vector.tensor_copy(
                dst, xp[:].rearrange("p (sb c) -> p sb c", sb=4))

        def xq_tr_q1(sb, eng):
            xp = pt_([P, 4 * P], "sc", 2, dt_=bf16)
            for t in range(4):
                cc = 4 + t
                nc.tensor.transpose(xp[:, t * P:(t + 1) * P],
                                    xt[:, cc * S + sb * P: cc * S + (sb + 1) * P],
                                    eye_bf)
            dst = xnat[:, sb * D + 4 * P: sb * D + 8 * P]
            (eng.tensor_copy if eng is nc.vector else eng.copy)(dst, xp[:])

        # ---------- fused softmax -> weights -> A -> graw -> extract -------
        def softmax_ws(w16, bias_ap, wout, wall, b_row_ap, mask3, g_out,
                       sh_outer):
            lgs = [pt_([16, SH], "lg", 4) for _ in range(NSH)]
            if sh_outer:
                for sh in range(NSH):
                    for cb in range(NCC):
                        nc.tensor.matmul(
                            lgs[sh][:], w16[:, cb * H:(cb + 1) * H],
                            xt[:, cb * S + sh * SH: cb * S + sh * SH + SH],
                            start=(cb == 0), stop=(cb == NCC - 1))
                    nc.scalar.activation(eE[:, sh * SH:(sh + 1) * SH],
                                         lgs[sh][:], AF.Exp,
                                         bias=bias_ap, scale=1.0)
            else:
                # cc-outer: consume xt chunks as the DMA lands them; the
                # first-quad xnat transposes ride along with their chunk
                for cb in range(NCC):
                    for sh in range(NSH):
                        nc.tensor.matmul(
                            lgs[sh][:], w16[:, cb * H:(cb + 1) * H],
                            xt[:, cb * S + sh * SH: cb * S + sh * SH + SH],
                            start=(cb == 0), stop=(cb == NCC - 1))
                    if cb < 4:
                        for g in range(4):
                            xq_tr_q0(cb, g)
                    else:
                        warm(1)
                for sh in range(NSH):
                    nc.scalar.activation(eE[:, sh * SH:(sh + 1) * SH],
                                         lgs[sh][:], AF.Exp,
                                         bias=bias_ap, scale=1.0)

            a0 = pt_([16, SH], "lg", 4)
            a1 = pt_([16, SH], "lg", 4)
            sps_t = pt_([P, SH], "big", 2)
            sps = sps_t[:16, :1]

            def a_mms(sb):
                lhs = wout[:, sb * H:(sb + 1) * H]
                nc.tensor.matmul(a0[:], lhs,
                                 xnat[:, sb * D: sb * D + SH],
                                 start=(sb == 0), stop=(sb == NSB - 1))
                nc.tensor.matmul(a1[:], lhs,
                                 xnat[:, sb * D + SH: sb * D + 2 * SH],
                                 start=(sb == 0), stop=(sb == NSB - 1))

            ngrp = NSB // NG
            first = not sh_outer
            for g in range(ngrp):
                eT_ps = pt_([P, 4 * P], "sc", 2, dt_=bf16)
                for t in range(NG):
                    sb = NG * g + t
                    nc.tensor.transpose(eT_ps[:, t * H:(t + 1) * H],
                                        eE[:, sb * P:(sb + 1) * P], id16)
                sl = slice(NG * g * H, (NG * g + NG) * H)
                gz = slice(NG * g, NG * g + NG)
                nc.vector.reduce_sum(
                    z_nat[:, gz].unsqueeze(2),
                    eT_ps[:, :NG * H].rearrange("p (sb h) -> p sb h", sb=NG),
                    axis=AXX)
                nc.vector.reciprocal(rz_nat[:, gz], z_nat[:, gz])
                nc.vector.tensor_tensor(
                    wout[:, sl].rearrange("p (sb h) -> p sb h", sb=NG),
                    eT_ps[:, :NG * H].rearrange("p (sb h) -> p sb h", sb=NG),
                    rz_nat[:, gz].unsqueeze(2).broadcast_to([P, NG, H]),
                    ALU.mult)
                # second-quad xnat transposes + previous group's A matmuls
                # fill PE while this group's exp/alpha chain completes
                if first:
                    for sb in range(NG * g, NG * g + NG):
                        xq_tr_q1(sb, nc.scalar if sb % 2 == 1 and sb < 8
                                 else nc.vector)
                if g > 0:
                    for sb in range(NG * (g - 1), NG * g):
                        a_mms(sb)
            for sb in range(NG * (ngrp - 1), NSB):
                a_mms(sb)
            for sb in range(NSB):
                nc.tensor.matmul(sps, wout[:, sb * H:(sb + 1) * H],
                                 ones_col[:],
                                 start=(sb == 0), stop=(sb == NSB - 1))

            # ---- A^T -> natural ----
            nc.scalar.copy(at_sb[:, :SH], a0[:])
            nc.vector.tensor_copy(at_sb[:, SH:], a1[:])
            nc.vector.tensor_copy(s_col[:], sps)
            trA = pt_([P, 4 * P], "sc", 2, dt_=bf16)
            for cc in range(NCC):
                nc.tensor.transpose(trA[:, cc * H:(cc + 1) * H],
                                    at_sb[:, cc * P:(cc + 1) * P], id16)
            nc.tensor.transpose(trA[:1, NSB * H:NSB * H + 16], s_col[:], id16)
            nc.vector.tensor_copy(a_nat[:], trA[:, :NCC * H])
            nc.vector.tensor_copy(s_row[:], trA[:1, NSB * H:NSB * H + 16])

            # ---- graw^T = A^T-stationary x W-moving (+ s b^T), then back ---
            gA = pt_([16, SH], "lg", 4)
            gB = pt_([16, SH], "lg", 4)
            for dd in range(NCC):
                nc.tensor.matmul(gA[:], a_nat[:, dd * H:(dd + 1) * H],
                                 wall[:, dd * D: dd * D + SH],
                                 start=(dd == 0), stop=False)
                nc.tensor.matmul(gB[:], a_nat[:, dd * H:(dd + 1) * H],
                                 wall[:, dd * D + SH: (dd + 1) * D],
                                 start=(dd == 0), stop=False)
            nc.tensor.matmul(gA[:], s_row[:1, :], b_row_ap[:1, :SH],
                             start=False, stop=True)
            nc.tensor.matmul(gB[:], s_row[:1, :], b_row_ap[:1, SH:],
                             start=False, stop=True)
            nc.scalar.copy(gts[:, :SH], gA[:])
            nc.vector.tensor_copy(gts[:, SH:], gB[:])
            grT = pt_([P, 4 * P], "sc", 2, dt_=bf16)
            for jb in range(NDB):
                nc.tensor.transpose(grT[:, jb * H:(jb + 1) * H],
                                    gts[:, jb * P:(jb + 1) * P], id16)
            nc.vector.tensor_tensor(ext_tmp[:], grT[:, :NDB * H], mask3[:],
                                    ALU.mult)
            nc.vector.reduce_sum(
                g_out[:].unsqueeze(2),
                ext_tmp[:].rearrange("p (j h) -> p j h", j=NDB),
                axis=AXX)

        # ---------- phase 1: alphas -> gq ----------
        warm(4)
        softmax_ws(wsm_sb, c0q_sb[:, :1], alpha, wqv_all, bqv_row, mask3S,
                   gq_sb, sh_outer=False)

        # ---------- phase 2: t = sc*diag(gq)*Wk_s ; Wfold ; betas -> gkd ----
        nc.vector.tensor_tensor(
            t_sb[:].rearrange("p (j h) -> p j h", j=NDB),
            wks_sb[:].rearrange("p (j h) -> p j h", j=NDB),
            gq_sb[:].unsqueeze(2).broadcast_to([P, NDB, H]),
            ALU.mult)
        c0k_t = pt_([P, SH], "big", 2)
        c0k_ps = c0k_t[:16, :1]
        for j in range(NDB):
            nc.tensor.matmul(c0k_ps, t_sb[:, j * H:(j + 1) * H],
                             bkpp[:, j:j + 1],
                             start=(j == 0), stop=(j == NDB - 1))
        nc.vector.tensor_tensor(c0k_sb[:], c0k_ps, bks_sb[:], ALU.add)
        # Wfold^T = t-stationary x WkT-moving (dd-progressive), then back
        wfA = pt_([16, SH], "lg", 4)
        wfB = pt_([16, SH], "lg", 4)
        for dd in range(NCC):
            nc.tensor.matmul(wfA[:], t_sb[:, dd * H:(dd + 1) * H],
                             wkT_all[:, dd * D: dd * D + SH],
                             start=(dd == 0), stop=(dd == NCC - 1))
            nc.tensor.matmul(wfB[:], t_sb[:, dd * H:(dd + 1) * H],
                             wkT_all[:, dd * D + SH: (dd + 1) * D],
                             start=(dd == 0), stop=(dd == NCC - 1))
        nc.scalar.copy(gts[:, :SH], wfA[:])
        nc.vector.tensor_copy(gts[:, SH:], wfB[:])
        wf_t = pt_([P, 4 * P], "sc", 2, dt_=bf16)
        for jb in range(NDB):
            nc.tensor.transpose(wf_t[:, jb * H:(jb + 1) * H],
                                gts[:, jb * P:(jb + 1) * P], id16)
        nc.vector.tensor_copy(wfold[:], wf_t[:, :NDB * H])
        softmax_ws(wfold, c0k_sb[:, :1], beta, wk_all, bk_row, mask3K,
                   gkd_sb, sh_outer=True)

        # ---------- phase 3: gk ; M = I + diag(gk) Wr ----------
        nc.vector.tensor_mul(gk_sb[:], gq_sb[:], gkd_sb[:])
        for cc in range(NCC):
            sl = slice(cc * D, (cc + 1) * D)
            if cc % 2 == 0:
                nc.vector.tensor_scalar(wr_all[:, sl], wr_all[:, sl],
                                        gk_sb[:, cc:cc + 1], None, ALU.mult)
            else:
                nc.scalar.activation(wr_all[:, sl], wr_all[:, sl], AF.Copy,
                                     bias=0.0, scale=gk_sb[:, cc:cc + 1])
            nc.vector.tensor_add(
                wr_all[:, cc * D + cc * P: cc * D + (cc + 1) * P],
                wr_all[:, cc * D + cc * P: cc * D + (cc + 1) * P], eye_bf)
        # ---------- phase 4: Wbig = W_qv @ M  (chases the per-chunk scales) -
        for cb in range(NCC):
            for eh in range(NDH):
                ps = pt_([P, SH], "big", 2)
                for dd in range(NCC):
                    nc.tensor.matmul(
                        ps[:], wqvT_all[:, dd * D + cb * P: dd * D + cb * P + P],
                        wr_all[:, dd * D + eh * SH: dd * D + (eh + 1) * SH],
                        start=(dd == 0), stop=(dd == NCC - 1))
                nc.scalar.copy(wbig[:, cb * D + eh * SH: cb * D + (eh + 1) * SH],
                               ps[:])

        # b_out = b_qv @ M + b_r
        for eh in range(NDH):
            bo = pt_([16, SH], "lg", 4)
            for j in range(NDB):
                nc.tensor.matmul(bo[:1, :], bqvpp[:, j:j + 1],
                                 wr_all[:, j * D + eh * SH: j * D + (eh + 1) * SH],
                                 start=(j == 0), stop=(j == NDB - 1))
            nc.vector.tensor_tensor(bout_sb[:1, eh * SH:(eh + 1) * SH],
                                    bo[:1, :], br_row[:1, eh * SH:(eh + 1) * SH],
                                    ALU.add)
        for eh in range(NDH):
            bb = pt_([P, SH], "big", 2)
            nc.tensor.matmul(bb[:], ones_row[:1, :],
                             bout_sb[:1, eh * SH:(eh + 1) * SH],
                             start=True, stop=True)
            nc.vector.tensor_copy(boutB[:, eh * SH:(eh + 1) * SH], bb[:])

        # ---------- phase 5: out = X @ Wbig + b_out ----------
        for sb in range(NSB):
            for eh in range(NDH):
                ps = pt_([P, SH], "big", 2)
                for cc in range(NCC):
                    nc.tensor.matmul(
                        ps[:], xt[:, cc * S + sb * P: cc * S + sb * P + P],
                        wbig[:, cc * D + eh * SH: cc * D + (eh + 1) * SH],
                        start=(cc == 0), stop=(cc == NCC - 1))
                if sb == NSB - 1:
                    hw = SH // 2
                    for q in range(2):
                        obq = st([P, hw], bf16, "obq", bufs=4)
                        nc.vector.tensor_tensor(
                            obq[:], ps[:, q * hw:(q + 1) * hw],
                            boutB[:, eh * SH + q * hw: eh * SH + (q + 1) * hw],
                            ALU.add)
                        nc.sync.dma_start(
                            OUT[sb * P:(sb + 1) * P,
                                eh * SH + q * hw: eh * SH + (q + 1) * hw],
                            obq[:])
                else:
                    ob = st([P, SH], bf16, "ob", bufs=4)
                    nc.vector.tensor_tensor(
                        ob[:], ps[:], boutB[:, eh * SH:(eh + 1) * SH], ALU.add)
                    eng = nc.gpsimd if sb < 12 and (sb * NDH + eh) % 2 == 0 \
                        else nc.sync
                    eng.dma_start(
                        OUT[sb * P:(sb + 1) * P, eh * SH:(eh + 1) * SH],
                        ob[:])

    nc.compile()
    return nc


def _get_nc():
    if "nc" not in _CACHE:
        _CACHE["nc"] = _build()
    return _CACHE["nc"]


def _prep_inputs(inputs):
    import ml_dtypes
    bf = ml_dtypes.bfloat16

    def f(k):
        return np.ascontiguousarray(np.asarray(inputs[k], dtype=np.float32))

    def c(a):
        return np.ascontiguousarray(np.asarray(a, dtype=np.float32).astype(bf))

    W_qv, W_k, W_r = f("W_qv"), f("W_k"), f("W_r")
    Wq_s, Wk_s = f("Wq_s"), f("Wk_s")
    b_qv, b_k, b_r = f("b_qv"), f("b_k"), f("b_r")
    bq_s, bk_s = f("bq_s"), f("bk_s")

    def perm(w):
        # [C*128, N] -> [128, C*N]: row-block cc goes to columns cc*N
        cb = w.shape[0] // P
        return w.reshape(cb, P, w.shape[1]).transpose(1, 0, 2).reshape(P, -1)

    # mask3[p, j*H + h] = v iff h == 2j + p//64
    pj = np.arange(P)[:, None] // 64 + 2 * np.arange(NDB)[None, :]  # [P, j]
    m3 = (pj[:, :, None] == np.arange(H)[None, None, :])            # [P, j, H]
    mask3S = (m3 * SCALE).reshape(P, NDB * H)
    mask3K = (m3 / SCALE).reshape(P, NDB * H)

    packW = np.concatenate([perm(SCALE * (W_qv @ Wq_s)), np.eye(P)], axis=1)
    packA = np.concatenate(
        [perm(Wk_s), b_qv.reshape(NDB, P).T, b_k.reshape(NDB, P).T,
         mask3S, mask3K], axis=1)
    packR = np.concatenate([b_qv, b_k]).reshape(1, 2 * D)

    common = {
        "Wqvb": c(perm(W_qv)), "WqvTb": c(perm(W_qv.T)),
        "Wkb": c(perm(W_k)), "WkTb": c(perm(W_k.T)),
        "Wrb": c(perm(W_r)),
        "packW": c(packW),
        "packA": c(packA),
        "packR": c(packR),
        "c0qf": np.ascontiguousarray(SCALE * (b_qv @ Wq_s + bq_s)),
        "bks_sf": np.ascontiguousarray(SCALE * bk_s),
        "br_f32": b_r,
    }
    in_maps = []
    for b in range(NCORES):
        m = dict(common)
        xb = np.asarray(inputs["X"][b], dtype=np.float32)
        m["XTb"] = c(perm(xb.T))
        in_maps.append(m)
    return in_maps


def run(inputs, trace=False):
    from concourse.bass_utils import run_bass_kernel_spmd

    nc = _get_nc()
    in_maps = _prep_inputs(inputs)
    res = run_bass_kernel_spmd(nc, in_maps, core_ids=list(range(NCORES)),
                               trace=trace)
    _CACHE["last_results"] = res
    out = np.stack([np.asarray(res.results[b]["out"], dtype=np.float32)
                    for b in range(NCORES)], axis=0)
    return out


def kernel(**inputs):
    trace = os.environ.get("KTRACE", "0") == "1"
    return run(inputs, trace=trace)


# revision 16
# speedup vs baseline: 1.0335x; 1.0038x over previous
"""AdditiveAttention distributed Bass kernel for 8 TRN2 NeuronCores.

Data-parallel over batch: B=8 samples -> 1 per core. Weights replicated.

Per-core math (S=2048, D=1024, H=16, HD=64, sc=1/sqrt(HD)):
  q = X @ W_qv + b_qv ; v = q ; k = X @ W_k + b_k
  alphas = softmax_h((q @ Wq_s + bq_s) * sc)
  gq[d]  = sum_s alphas[s, h(d)] * q[s, d]          h(d) = d // 64
  betas  = softmax_h(((k*gq) @ Wk_s + bk_s) * sc)
  gk[d]  = gq[d] * sum_s betas[s, h(d)] * k[s, d]
  out    = q + (q*gk) @ W_r + b_r

v3 restructure (kept): never materialize q or k; everything is X-based until
one fused output GEMM:
  logits_q^T = Wsm^T X^T,  Wsm = sc*(W_qv Wq_s)  (host),  + c0q bias in exp
  gq: A = X^T alpha, graw^T = A^T W_qv + s b^T, gq = masked diag extract
  logits_b^T = Wfold^T X^T, Wfold = W_k (sc*diag(gq) Wk_s)
  gk analogous via W_k and A_k
  out = X @ Wbig + b_out,  Wbig = W_qv (I + diag(gk) W_r),  b_out = b_qv@M + b_r

v5: the measured DMA bus is ~335 GB/s aggregate (~160 per queue), which makes
the input stream the long pole; and LoadStationary (~100ns, hidden only under
wide moving operands) dominates any matmul with a skinny moving side.
  - X natural is NOT shipped: xnat is derived from X^T by PE transposes that
    fill the DMA-wait bubbles (saves 4MB = ~12us of stream).
  - identity/masks ship from the host inside the wsm/packA transfers: gpsimd
    runs nothing but DMA issues (its affine_selects were serialized behind
    the SWDGE queue and stalled the first softmax by 9us).
  - graw and Wfold run in the moving-bound orientation (stationary = the
    [128,16] A^T / t chunks, moving = the 512-wide weight) instead of 64
    stationary reloads each; results transposed back on PE.
  - softmax+A pipelined per 4-s-block group; softmax2 sh-outer; warm-up
    matmuls keep the PE p-state up during DMA waits; fold chases per-chunk
    diag(gk) scales; gpsimd carries no late output tiles (drain tail).

All matmuls bf16 with f32 PSUM. Output stored bf16, host upcasts.
"""

import math
import os
from contextlib import ExitStack

import numpy as np

B, S, D, H = 8, 2048, 1024, 16
HD = D // H
SCALE = 1.0 / math.sqrt(HD)
NCORES = 8
P = 128
NDB = D // P      # 8 d-blocks
NSB = S // P      # 16 s-blocks
NCC = D // P      # 8 contraction chunks
SH = 512
NSH = S // SH     # 4
NDH = D // SH     # 2
NG = 4            # s-block group size for softmax pipelining

_CACHE = {}


def _build():
    import concourse.bacc as bacc
    import concourse.tile as tile
    import concourse.mybir as mybir

    f32 = mybir.dt.float32
    bf16 = mybir.dt.bfloat16
    AF = mybir.ActivationFunctionType
    ALU = mybir.AluOpType
    AXX = mybir.AxisListType.X

    nc = bacc.Bacc("TRN2", target_bir_lowering=False, debug=False,
                   num_devices=NCORES)

    # bulk tensors HOST-PRE-PERMUTED into SBUF layout [128, N]
    XT = nc.dram_tensor("XTb", [P, NCC * S], bf16, kind="ExternalInput").ap()
    Wqv = nc.dram_tensor("Wqvb", [P, NCC * D], bf16, kind="ExternalInput").ap()
    WqvT = nc.dram_tensor("WqvTb", [P, NCC * D], bf16, kind="ExternalInput").ap()
    Wk = nc.dram_tensor("Wkb", [P, NCC * D], bf16, kind="ExternalInput").ap()
    WkT = nc.dram_tensor("WkTb", [P, NCC * D], bf16, kind="ExternalInput").ap()
    Wr = nc.dram_tensor("Wrb", [P, NCC * D], bf16, kind="ExternalInput").ap()
    # packW = Wsm | eye128   (id16 = eye[:16,:16])
    PackW = nc.dram_tensor("packW", [P, NDB * H + P], bf16,
                           kind="ExternalInput").ap()
    # packA = Wks | bqvpp | bkpp | mask3S | mask3K
    NPA = NDB * H + 2 * NDB + 2 * NDB * H
    PackA = nc.dram_tensor("packA", [P, NPA], bf16, kind="ExternalInput").ap()
    # packR = bqv_row | bk_row
    PackR = nc.dram_tensor("packR", [1, 2 * D], bf16, kind="ExternalInput").ap()
    c0q = nc.dram_tensor("c0qf", [H], f32, kind="ExternalInput").ap()
    bks_s = nc.dram_tensor("bks_sf", [H], f32, kind="ExternalInput").ap()
    br_f = nc.dram_tensor("br_f32", [D], f32, kind="ExternalInput").ap()
    OUT = nc.dram_tensor("out", [S, D], bf16, kind="ExternalOutput").ap()

    with tile.TileContext(nc) as tc, ExitStack() as ctx:
        sbp = ctx.enter_context(tc.tile_pool(name="sbp", bufs=1))
        psp = ctx.enter_context(tc.tile_pool(name="psp", bufs=1, space="PSUM"))

        def st(shape, dt_, tag, bufs=1):
            return sbp.tile(shape, dt_, tag=tag, bufs=bufs, name=tag)

        def pt_(shape, tag, bufs, dt_=f32):
            return psp.tile(shape, dt_, tag=tag, bufs=bufs, name=tag)

        # ---------- resident big tensors ----------
        xt = st([P, NCC * S], bf16, "xt")       # X^T, chunk cc at cols cc*S
        xnat = st([P, NSB * D], bf16, "xnat")   # X natural (PE-derived)
        wqv_all = st([P, NCC * D], bf16, "wqv_all")
        wqvT_all = st([P, NCC * D], bf16, "wqvT_all")
        wk_all = st([P, NCC * D], bf16, "wk_all")
        wkT_all = st([P, NCC * D], bf16, "wkT_all")
        wr_all = st([P, NCC * D], bf16, "wr_all")   # becomes M = I+diag(gk)Wr
        wbig = st([P, NCC * D], bf16, "wbig")

        # ---------- small persistent ----------
        packW_sb = st([P, NDB * H + P], bf16, "packW_sb")
        wsm_sb = packW_sb[:, :NDB * H]
        eye_bf = packW_sb[:, NDB * H:]
        id16 = packW_sb[:16, NDB * H:NDB * H + 16]
        packA_sb = st([P, NPA], bf16, "packA_sb")
        wks_sb = packA_sb[:, :NDB * H]
        bqvpp = packA_sb[:, NDB * H:NDB * H + NDB]
        bkpp = packA_sb[:, NDB * H + NDB:NDB * H + 2 * NDB]
        mask3S = packA_sb[:, NDB * H + 2 * NDB:2 * NDB * H + 2 * NDB]
        mask3K = packA_sb[:, 2 * NDB * H + 2 * NDB:]
        packR_sb = st([1, 2 * D], bf16, "packR_sb")
        bqv_row = packR_sb[:1, :D]
        bk_row = packR_sb[:1, D:]
        t_sb = st([P, NDB * H], bf16, "t_sb")
        wfold = st([P, NDB * H], bf16, "wfold")
        c0q_sb = st([16, 1], f32, "c0q_sb")
        bks_sb = st([16, 1], f32, "bks_sb")
        c0k_sb = st([16, 1], f32, "c0k_sb")
        br_row = st([1, D], f32, "br_row")
        bout_sb = st([1, D], bf16, "bout_sb")
        boutB = st([P, D], f32, "boutB")

        eE = st([16, S], bf16, "eE")
        z_nat = st([P, NSB], f32, "z_nat")
        rz_nat = st([P, NSB], f32, "rz_nat")
        alpha = st([P, NSB * H], bf16, "alpha")
        beta = st([P, NSB * H], bf16, "beta")
        at_sb = st([16, D], bf16, "at_sb")
        gts = st([16, D], bf16, "gts")          # graw^T / Wfold^T staging
        a_nat = st([P, NCC * H], bf16, "a_nat")
        s_col = st([16, 1], bf16, "s_col")
        s_row = st([1, 16], bf16, "s_row")
        ext_tmp = st([P, NDB * H], f32, "ext_tmp")
        gq_sb = st([P, NDB], f32, "gq_sb")      # = SCALE * gq
        gkd_sb = st([P, NDB], f32, "gkd_sb")    # = gkd / SCALE
        gk_sb = st([P, NDB], f32, "gk_sb")

        # ---------- small DMAs on scalar (idle until the first exps) -------
        nc.scalar.dma_start(packW_sb[:], PackW[:, :])
        nc.scalar.dma_start(c0q_sb[:], c0q.unsqueeze(1))
        nc.scalar.dma_start(bks_sb[:], bks_s.unsqueeze(1))
        nc.scalar.dma_start(packR_sb[:], PackR[:, :])
        nc.scalar.dma_start(packA_sb[:], PackA[:, :])
        nc.scalar.dma_start(br_row[:], br_f.unsqueeze(0))

        # ---------- bulk DMA: stripe BOTH queues, global first-need order ---
        # Each queue's steady state is ~2.1us fixed per transfer (semaphore
        # grant + reissue) plus wire time, so 512KB transfers crawl at
        # ~155GB/s while the burst rate is ~430GB/s.  1MB transfers amortize
        # the overhead to ~230GB/s/queue; two queues then reach the HBM cap.
        def xt_pair(cc, eng):
            eng.dma_start(xt[:, cc * S:(cc + 2) * S],
                          XT[:, cc * S:(cc + 2) * S])

        def w_half(dst, src_, half, eng):
            lo = half * (NCC // 2)
            eng.dma_start(dst[:, lo * D:(lo + NCC // 2) * D],
                          src_[:, lo * D:(lo + NCC // 2) * D])

        xt_pair(0, nc.sync)
        xt_pair(4, nc.gpsimd)
        xt_pair(2, nc.sync)
        xt_pair(6, nc.gpsimd)
        for src_, dst in ((Wqv, wqv_all), (WkT, wkT_all), (Wk, wk_all),
                          (Wr, wr_all), (WqvT, wqvT_all)):
            w_half(dst, src_, 0, nc.sync)
            w_half(dst, src_, 1, nc.gpsimd)

        # ---------- tiny constants on vector ----------
        ones_col = st([P, 1], bf16, "ones_col")
        nc.vector.memset(ones_col[:], 1.0)
        ones_row = st([1, P], bf16, "ones_row")
        nc.vector.memset(ones_row[:], 1.0)

        # ---------- PE warm-up: junk matmuls on packW while DMA streams ----
        def warm(n):
            for _ in range(n):
                wt = pt_([P, SH], "big", 2)
                nc.tensor.matmul(wt[:, :P], packW_sb[:, :P], packW_sb[:, :P],
                                 start=True, stop=True)

        # ---------- xnat derivation: xt chunk cc -> transposed s-blocks ----
        # quad0 (cc 0..3): cc-major as the chunks land; xp tile holds 4
        # s-blocks of one cc, evacuated with a strided copy.
        # quad1 (cc 4..7): sb-major so A(sb) unblocks in order; contiguous.
        def xq_tr_q0(cc, g):
            xp = pt_([P, 4 * P], "sc", 2, dt_=bf16)
            for t in range(4):
                sb = 4 * g + t
                nc.tensor.transpose(xp[:, t * P:(t + 1) * P],
                                    xt[:, cc * S + sb * P: cc * S + (sb + 1) * P],
                                    eye_bf)
            dst = xnat[:].rearrange("p (sb c) -> p sb c", sb=NSB)[
                :, 4 * g:4 * g + 4, cc * P:(cc + 1) * P]
            nc.vector.tensor_copy(
                dst, xp[:].rearrange("p (sb c) -> p sb c", sb=4))

        def xq_tr_q1(sb, eng):
            xp = pt_([P, 4 * P], "sc", 2, dt_=bf16)
            for t in range(4):
                cc = 4 + t
                nc.tensor.transpose(xp[:, t * P:(t + 1) * P],
                                    xt[:, cc * S + sb * P: cc * S + (sb + 1) * P],
                                    eye_bf)
            dst = xnat[:, sb * D + 4 * P: sb * D + 8 * P]
            (eng.tensor_copy if eng is nc.vector else eng.copy)(dst, xp[:])

        # ---------- fused softmax -> weights -> A -> graw -> extract -------
        def softmax_ws(w16, bias_ap, wout, wall, b_row_ap, mask3, g_out,
                       sh_outer):
            lgs = [pt_([16, SH], "lg", 4) for _ in range(NSH)]
            if sh_outer:
                for sh in range(NSH):
                    for cb in range(NCC):
                        nc.tensor.matmul(
                            lgs[sh][:], w16[:, cb * H:(cb + 1) * H],
                            xt[:, cb * S + sh * SH: cb * S + sh * SH + SH],
                            start=(cb == 0), stop=(cb == NCC - 1))
                    nc.scalar.activation(eE[:, sh * SH:(sh + 1) * SH],
                                         lgs[sh][:], AF.Exp,
                                         bias=bias_ap, scale=1.0)
            else:
                # cc-outer: consume xt chunks as the DMA lands them; the
                # first-quad xnat transposes ride along with their chunk
                for cb in range(NCC):
                    for sh in range(NSH):
                        nc.tensor.matmul(
                            lgs[sh][:], w16[:, cb * H:(cb + 1) * H],
                            xt[:, cb * S + sh * SH: cb * S + sh * SH + SH],
                            start=(cb == 0), stop=(cb == NCC - 1))
                    if cb < 4:
                        for g in range(4):
                            xq_tr_q0(cb, g)
                    else:
                        warm(1)
                for sh in range(NSH):
                    nc.scalar.activation(eE[:, sh * SH:(sh + 1) * SH],
                                         lgs[sh][:], AF.Exp,
                                         bias=bias_ap, scale=1.0)

            a0 = pt_([16, SH], "lg", 4)
            a1 = pt_([16, SH], "lg", 4)
            sps_t = pt_([P, SH], "big", 2)
            sps = sps_t[:16, :1]

            def a_mms(sb):
                lhs = wout[:, sb * H:(sb + 1) * H]
                nc.tensor.matmul(a0[:], lhs,
                                 xnat[:, sb * D: sb * D + SH],
                                 start=(sb == 0), stop=(sb == NSB - 1))
                nc.tensor.matmul(a1[:], lhs,
                                 xnat[:, sb * D + SH: sb * D + 2 * SH],
                                 start=(sb == 0), stop=(sb == NSB - 1))

            ngrp = NSB // NG
            first = not sh_outer
            for g in range(ngrp):
                eT_ps = pt_([P, 4 * P], "sc", 2, dt_=bf16)
                for t in range(NG):
                    sb = NG * g + t
                    nc.tensor.transpose(eT_ps[:, t * H:(t + 1) * H],
                                        eE[:, sb * P:(sb + 1) * P], id16)
                sl = slice(NG * g * H, (NG * g + NG) * H)
                gz = slice(NG * g, NG * g + NG)
                nc.vector.reduce_sum(
                    z_nat[:, gz].unsqueeze(2),
                    eT_ps[:, :NG * H].rearrange("p (sb h) -> p sb h", sb=NG),
                    axis=AXX)
                nc.vector.reciprocal(rz_nat[:, gz], z_nat[:, gz])
                nc.vector.tensor_tensor(
                    wout[:, sl].rearrange("p (sb h) -> p sb h", sb=NG),
                    eT_ps[:, :NG * H].rearrange("p (sb h) -> p sb h", sb=NG),
                    rz_nat[:, gz].unsqueeze(2).broadcast_to([P, NG, H]),
                    ALU.mult)
                # second-quad xnat transposes + previous group's A matmuls
                # fill PE while this group's exp/alpha chain completes
                if first:
                    for sb in range(NG * g, NG * g + NG):
                        xq_tr_q1(sb, nc.scalar if sb % 2 == 1 and sb < 8
                                 else nc.vector)
                if g > 0:
                    for sb in range(NG * (g - 1), NG * g):
                        a_mms(sb)
            for sb in range(NG * (ngrp - 1), NSB):
                a_mms(sb)
            for sb in range(NSB):
                nc.tensor.matmul(sps, wout[:, sb * H:(sb + 1) * H],
                                 ones_col[:],
                                 start=(sb == 0), stop=(sb == NSB - 1))

            # ---- A^T -> natural ----
            nc.scalar.copy(at_sb[:, :SH], a0[:])
            nc.vector.tensor_copy(at_sb[:, SH:], a1[:])
            nc.vector.tensor_copy(s_col[:], sps)
            trA = pt_([P, 4 * P], "sc", 2, dt_=bf16)
            for cc in range(NCC):
                nc.tensor.transpose(trA[:, cc * H:(cc + 1) * H],
                                    at_sb[:, cc * P:(cc + 1) * P], id16)
            nc.tensor.transpose(trA[:1, NSB * H:NSB * H + 16], s_col[:], id16)
            nc.vector.tensor_copy(a_nat[:], trA[:, :NCC * H])
            nc.vector.tensor_copy(s_row[:], trA[:1, NSB * H:NSB * H + 16])

            # ---- graw^T = A^T-stationary x W-moving (+ s b^T), then back ---
            gA = pt_([16, SH], "lg", 4)
            gB = pt_([16, SH], "lg", 4)
            for dd in range(NCC):
                nc.tensor.matmul(gA[:], a_nat[:, dd * H:(dd + 1) * H],
                                 wall[:, dd * D: dd * D + SH],
                                 start=(dd == 0), stop=False)
                nc.tensor.matmul(gB[:], a_nat[:, dd * H:(dd + 1) * H],
                                 wall[:, dd * D + SH: (dd + 1) * D],
                                 start=(dd == 0), stop=False)
            nc.tensor.matmul(gA[:], s_row[:1, :], b_row_ap[:1, :SH],
                             start=False, stop=True)
            nc.tensor.matmul(gB[:], s_row[:1, :], b_row_ap[:1, SH:],
                             start=False, stop=True)
            nc.scalar.copy(gts[:, :SH], gA[:])
            nc.vector.tensor_copy(gts[:, SH:], gB[:])
            grT = pt_([P, 4 * P], "sc", 2, dt_=bf16)
            for jb in range(NDB):
                nc.tensor.transpose(grT[:, jb * H:(jb + 1) * H],
                                    gts[:, jb * P:(jb + 1) * P], id16)
            nc.vector.tensor_tensor(ext_tmp[:], grT[:, :NDB * H], mask3[:],
                                    ALU.mult)
            nc.vector.reduce_sum(
                g_out[:].unsqueeze(2),
                ext_tmp[:].rearrange("p (j h) -> p j h", j=NDB),
                axis=AXX)

        # ---------- phase 1: alphas -> gq ----------
        warm(4)
        softmax_ws(wsm_sb, c0q_sb[:, :1], alpha, wqv_all, bqv_row, mask3S,
                   gq_sb, sh_outer=False)

        # ---------- phase 2: t = sc*diag(gq)*Wk_s ; Wfold ; betas -> gkd ----
        nc.vector.tensor_tensor(
            t_sb[:].rearrange("p (j h) -> p j h", j=NDB),
            wks_sb[:].rearrange("p (j h) -> p j h", j=NDB),
            gq_sb[:].unsqueeze(2).broadcast_to([P, NDB, H]),
            ALU.mult)
        c0k_t = pt_([P, SH], "big", 2)
        c0k_ps = c0k_t[:16, :1]
        for j in range(NDB):
            nc.tensor.matmul(c0k_ps, t_sb[:, j * H:(j + 1) * H],
                             bkpp[:, j:j + 1],
                             start=(j == 0), stop=(j == NDB - 1))
        nc.vector.tensor_tensor(c0k_sb[:], c0k_ps, bks_sb[:], ALU.add)
        # Wfold^T = t-stationary x WkT-moving (dd-progressive), then back
        wfA = pt_([16, SH], "lg", 4)
        wfB = pt_([16, SH], "lg", 4)
        for dd in range(NCC):
            nc.tensor.matmul(wfA[:], t_sb[:, dd * H:(dd + 1) * H],
                             wkT_all[:, dd * D: dd * D + SH],
                             start=(dd == 0), stop=(dd == NCC - 1))
            nc.tensor.matmul(wfB[:], t_sb[:, dd * H:(dd + 1) * H],
                             wkT_all[:, dd * D + SH: (dd + 1) * D],
                             start=(dd == 0), stop=(dd == NCC - 1))
        nc.scalar.copy(gts[:, :SH], wfA[:])
        nc.vector.tensor_copy(gts[:, SH:], wfB[:])
        wf_t = pt_([P, 4 * P], "sc", 2, dt_=bf16)
        for jb in range(NDB):
            nc.tensor.transpose(wf_t[:, jb * H:(jb + 1) * H],
                                gts[:, jb * P:(jb + 1) * P], id16)
        nc.vector.tensor_copy(wfold[:], wf_t[:, :NDB * H])
        softmax_ws(wfold, c0k_sb[:, :1], beta, wk_all, bk_row, mask3K,
                   gkd_sb, sh_outer=True)

        # ---------- phase 3: gk ; M = I + diag(gk) Wr ----------
        nc.vector.tensor_mul(gk_sb[:], gq_sb[:], gkd_sb[:])
        for cc in range(NCC):
            sl = slice(cc * D, (cc + 1) * D)
            if cc % 2 == 0:
                nc.vector.tensor_scalar(wr_all[:, sl], wr_all[:, sl],
                                        gk_sb[:, cc:cc + 1], None, ALU.mult)
            else:
                nc.scalar.activation(wr_all[:, sl], wr_all[:, sl], AF.Copy,
                                     bias=0.0, scale=gk_sb[:, cc:cc + 1])
            nc.vector.tensor_add(
                wr_all[:, cc * D + cc * P: cc * D + (cc + 1) * P],
                wr_all[:, cc * D + cc * P: cc * D + (cc + 1) * P], eye_bf)
        # ---------- phase 4: Wbig = W_qv @ M  (chases the per-chunk scales) -
        for cb in range(NCC):
            for eh in range(NDH):
                ps = pt_([P, SH], "big", 2)
                for dd in range(NCC):
                    nc.tensor.matmul(
                        ps[:], wqvT_all[:, dd * D + cb * P: dd * D + cb * P + P],
                        wr_all[:, dd * D + eh * SH: dd * D + (eh + 1) * SH],
                        start=(dd == 0), stop=(dd == NCC - 1))
                nc.scalar.copy(wbig[:, cb * D + eh * SH: cb * D + (eh + 1) * SH],
                               ps[:])

        # b_out = b_qv @ M + b_r
        for eh in range(NDH):
            bo = pt_([16, SH], "lg", 4)
            for j in range(NDB):
                nc.tensor.matmul(bo[:1, :], bqvpp[:, j:j + 1],
                                 wr_all[:, j * D + eh * SH: j * D + (eh + 1) * SH],
                                 start=(j == 0), stop=(j == NDB - 1))
            nc.vector.tensor_tensor(bout_sb[:1, eh * SH:(eh + 1) * SH],
                                    bo[:1, :], br_row[:1, eh * SH:(eh + 1) * SH],
                                    ALU.add)
        for eh in range(NDH):
            bb = pt_([P, SH], "big", 2)
            nc.tensor.matmul(bb[:], ones_row[:1, :],
                             bout_sb[:1, eh * SH:(eh + 1) * SH],
                             start=True, stop=True)
            nc.vector.tensor_copy(boutB[:, eh * SH:(eh + 1) * SH], bb[:])

        # ---------- phase 5: out = X @ Wbig + b_out ----------
        for sb in range(NSB):
            for eh in range(NDH):
                ps = pt_([P, SH], "big", 2)
                for cc in range(NCC):
                    nc.tensor.matmul(
                        ps[:], xt[:, cc * S + sb * P: cc * S + sb * P + P],
                        wbig[:, cc * D + eh * SH: cc * D + (eh + 1) * SH],
                        start=(cc == 0), stop=(cc == NCC - 1))
                if sb == NSB - 1:
                    hw = SH // 2
                    for q in range(2):
                        obq = st([P, hw], bf16, "obq", bufs=4)
                        nc.vector.tensor_tensor(
                            obq[:], ps[:, q * hw:(q + 1) * hw],
                            boutB[:, eh * SH + q * hw: eh * SH + (q + 1) * hw],
                            ALU.add)
                        nc.sync.dma_start(
                            OUT[sb * P:(sb + 1) * P,
                                eh * SH + q * hw: eh * SH + (q + 1) * hw],
                            obq[:])
                else:
                    ob = st([P, SH], bf16, "ob", bufs=4)
                    nc.vector.tensor_tensor(
                        ob[:], ps[:], boutB[:, eh * SH:(eh + 1) * SH], ALU.add)
                    eng = nc.gpsimd if sb < 12 and (sb * NDH + eh) % 2 == 0 \
                        else nc.sync
                    eng.dma_start(
                        OUT[sb * P:(sb + 1) * P, eh * SH:(eh + 1) * SH],
                        ob[:])

    nc.compile()
    return nc


def _get_nc():
    if "nc" not in _CACHE:
        _CACHE["nc"] = _build()
    return _CACHE["nc"]


def _prep_inputs(inputs):
    import ml_dtypes
    bf = ml_dtypes.bfloat16

    def f(k):
        return np.ascontiguousarray(np.asarray(inputs[k], dtype=np.float32))

    def c(a):
        return np.ascontiguousarray(np.asarray(a, dtype=np.float32).astype(bf))

    W_qv, W_k, W_r = f("W_qv"), f("W_k"), f("W_r")
    Wq_s, Wk_s = f("Wq_s"), f("Wk_s")
    b_qv, b_k, b_r = f("b_qv"), f("b_k"), f("b_r")
    bq_s, bk_s = f("bq_s"), f("bk_s")

    def perm(w):
        # [C*128, N] -> [128, C*N]: row-block cc goes to columns cc*N
        cb = w.shape[0] // P
        return w.reshape(cb, P, w.shape[1]).transpose(1, 0, 2).reshape(P, -1)

    # mask3[p, j*H + h] = v iff h == 2j + p//64
    pj = np.arange(P)[:, None] // 64 + 2 * np.arange(NDB)[None, :]  # [P, j]
    m3 = (pj[:, :, None] == np.arange(H)[None, None, :])            # [P, j, H]
    mask3S = (m3 * SCALE).reshape(P, NDB * H)
    mask3K = (m3 / SCALE).reshape(P, NDB * H)

    packW = np.concatenate([perm(SCALE * (W_qv @ Wq_s)), np.eye(P)], axis=1)
    packA = np.concatenate(
        [perm(Wk_s), b_qv.reshape(NDB, P).T, b_k.reshape(NDB, P).T,
         mask3S, mask3K], axis=1)
    packR = np.concatenate([b_qv, b_k]).reshape(1, 2 * D)

    common = {
        "Wqvb": c(perm(W_qv)), "WqvTb": c(perm(W_qv.T)),
        "Wkb": c(perm(W_k)), "WkTb": c(perm(W_k.T)),
        "Wrb": c(perm(W_r)),
        "packW": c(packW),
        "packA": c(packA),
        "packR": c(packR),
        "c0qf": np.ascontiguousarray(SCALE * (b_qv @ Wq_s + bq_s)),
        "bks_sf": np.ascontiguousarray(SCALE * bk_s),
        "br_f32": b_r,
    }
    in_maps = []
    for b in range(NCORES):
        m = dict(common)
        xb = np.asarray(inputs["X"][b], dtype=np.float32)
        m["XTb"] = c(perm(xb.T))
        in_maps.append(m)
    return in_maps


def run(inputs, trace=False):
    from concourse.bass_utils import run_bass_kernel_spmd

    nc = _get_nc()
    in_maps = _prep_inputs(inputs)
    res = run_bass_kernel_spmd(nc, in_maps, core_ids=list(range(NCORES)),
                               trace=trace)
    _CACHE["last_results"] = res
    out = np.stack([np.asarray(res.results[b]["out"], dtype=np.float32)
                    for b in range(NCORES)], axis=0)
    return out


def kernel(**inputs):
    trace = os.environ.get("KTRACE", "0") == "1"
    return run(inputs, trace=trace)


# revision 17
# speedup vs baseline: 1.0491x; 1.0151x over previous
"""AdditiveAttention distributed Bass kernel for 8 TRN2 NeuronCores.

Data-parallel over batch: B=8 samples -> 1 per core. Weights replicated.

Per-core math (S=2048, D=1024, H=16, HD=64, sc=1/sqrt(HD)):
  q = X @ W_qv + b_qv ; v = q ; k = X @ W_k + b_k
  alphas = softmax_h((q @ Wq_s + bq_s) * sc)
  gq[d]  = sum_s alphas[s, h(d)] * q[s, d]          h(d) = d // 64
  betas  = softmax_h(((k*gq) @ Wk_s + bk_s) * sc)
  gk[d]  = gq[d] * sum_s betas[s, h(d)] * k[s, d]
  out    = q + (q*gk) @ W_r + b_r

v3 restructure (kept): never materialize q or k; everything is X-based until
one fused output GEMM:
  logits_q^T = Wsm^T X^T,  Wsm = sc*(W_qv Wq_s)  (host),  + c0q bias in exp
  gq: A = X^T alpha, graw^T = A^T W_qv + s b^T, gq = masked diag extract
  logits_b^T = Wfold^T X^T, Wfold = W_k (sc*diag(gq) Wk_s)
  gk analogous via W_k and A_k
  out = X @ Wbig + b_out,  Wbig = W_qv (I + diag(gk) W_r),  b_out = b_qv@M + b_r

v5: the measured DMA bus is ~335 GB/s aggregate (~160 per queue), which makes
the input stream the long pole; and LoadStationary (~100ns, hidden only under
wide moving operands) dominates any matmul with a skinny moving side.
  - X natural is NOT shipped: xnat is derived from X^T by PE transposes that
    fill the DMA-wait bubbles (saves 4MB = ~12us of stream).
  - identity/masks ship from the host inside the wsm/packA transfers: gpsimd
    runs nothing but DMA issues (its affine_selects were serialized behind
    the SWDGE queue and stalled the first softmax by 9us).
  - graw and Wfold run in the moving-bound orientation (stationary = the
    [128,16] A^T / t chunks, moving = the 512-wide weight) instead of 64
    stationary reloads each; results transposed back on PE.
  - softmax+A pipelined per 4-s-block group; softmax2 sh-outer; warm-up
    matmuls keep the PE p-state up during DMA waits; fold chases per-chunk
    diag(gk) scales; gpsimd carries no late output tiles (drain tail).

All matmuls bf16 with f32 PSUM. Output stored bf16, host upcasts.
"""

import math
import os
from contextlib import ExitStack

import numpy as np

B, S, D, H = 8, 2048, 1024, 16
HD = D // H
SCALE = 1.0 / math.sqrt(HD)
NCORES = 8
P = 128
NDB = D // P      # 8 d-blocks
NSB = S // P      # 16 s-blocks
NCC = D // P      # 8 contraction chunks
SH = 512
NSH = S // SH     # 4
NDH = D // SH     # 2
NG = 4            # s-block group size for softmax pipelining

_CACHE = {}


def _build():
    import concourse.bacc as bacc
    import concourse.tile as tile
    import concourse.mybir as mybir

    f32 = mybir.dt.float32
    bf16 = mybir.dt.bfloat16
    AF = mybir.ActivationFunctionType
    ALU = mybir.AluOpType
    AXX = mybir.AxisListType.X

    nc = bacc.Bacc("TRN2", target_bir_lowering=False, debug=False,
                   num_devices=NCORES)

    # bulk tensors HOST-PRE-PERMUTED into SBUF layout [128, N]
    X = nc.dram_tensor("Xb", [P, NSB * D], bf16, kind="ExternalInput").ap()
    XT = nc.dram_tensor("XTb", [P, NCC * S], bf16, kind="ExternalInput").ap()
    Wqv = nc.dram_tensor("Wqvb", [P, NCC * D], bf16, kind="ExternalInput").ap()
    WqvT = nc.dram_tensor("WqvTb", [P, NCC * D], bf16, kind="ExternalInput").ap()
    Wk = nc.dram_tensor("Wkb", [P, NCC * D], bf16, kind="ExternalInput").ap()
    WkT = nc.dram_tensor("WkTb", [P, NCC * D], bf16, kind="ExternalInput").ap()
    Wr = nc.dram_tensor("Wrb", [P, NCC * D], bf16, kind="ExternalInput").ap()
    # packW = Wsm | eye128   (id16 = eye[:16,:16])
    PackW = nc.dram_tensor("packW", [P, NDB * H + P], bf16,
                           kind="ExternalInput").ap()
    # packA = Wks | bqvpp | bkpp | mask3S | mask3K
    NPA = NDB * H + 2 * NDB + 2 * NDB * H
    PackA = nc.dram_tensor("packA", [P, NPA], bf16, kind="ExternalInput").ap()
    # packR = bqv_row | bk_row
    PackR = nc.dram_tensor("packR", [1, 2 * D], bf16, kind="ExternalInput").ap()
    c0q = nc.dram_tensor("c0qf", [H], f32, kind="ExternalInput").ap()
    bks_s = nc.dram_tensor("bks_sf", [H], f32, kind="ExternalInput").ap()
    br_f = nc.dram_tensor("br_f32", [D], f32, kind="ExternalInput").ap()
    OUT = nc.dram_tensor("out", [S, D], bf16, kind="ExternalOutput").ap()

    with tile.TileContext(nc) as tc, ExitStack() as ctx:
        sbp = ctx.enter_context(tc.tile_pool(name="sbp", bufs=1))
        psp = ctx.enter_context(tc.tile_pool(name="psp", bufs=1, space="PSUM"))

        def st(shape, dt_, tag, bufs=1):
            return sbp.tile(shape, dt_, tag=tag, bufs=bufs, name=tag)

        def pt_(shape, tag, bufs, dt_=f32):
            return psp.tile(shape, dt_, tag=tag, bufs=bufs, name=tag)

        # ---------- resident big tensors ----------
        xt = st([P, NCC * S], bf16, "xt")       # X^T, chunk cc at cols cc*S
        xnat = st([P, NSB * D], bf16, "xnat")   # X natural, s-block si at si*D
        wqv_all = st([P, NCC * D], bf16, "wqv_all")
        wqvT_all = st([P, NCC * D], bf16, "wqvT_all")
        wk_all = st([P, NCC * D], bf16, "wk_all")
        wkT_all = st([P, NCC * D], bf16, "wkT_all")
        wr_all = st([P, NCC * D], bf16, "wr_all")   # becomes M = I+diag(gk)Wr
        wbig = st([P, NCC * D], bf16, "wbig")

        # ---------- small persistent ----------
        packW_sb = st([P, NDB * H + P], bf16, "packW_sb")
        wsm_sb = packW_sb[:, :NDB * H]
        eye_bf = packW_sb[:, NDB * H:]
        id16 = packW_sb[:16, NDB * H:NDB * H + 16]
        packA_sb = st([P, NPA], bf16, "packA_sb")
        wks_sb = packA_sb[:, :NDB * H]
        bqvpp = packA_sb[:, NDB * H:NDB * H + NDB]
        bkpp = packA_sb[:, NDB * H + NDB:NDB * H + 2 * NDB]
        mask3S = packA_sb[:, NDB * H + 2 * NDB:2 * NDB * H + 2 * NDB]
        mask3K = packA_sb[:, 2 * NDB * H + 2 * NDB:]
        packR_sb = st([1, 2 * D], bf16, "packR_sb")
        bqv_row = packR_sb[:1, :D]
        bk_row = packR_sb[:1, D:]
        t_sb = st([P, NDB * H], bf16, "t_sb")
        wfold = st([P, NDB * H], bf16, "wfold")
        c0q_sb = st([16, 1], f32, "c0q_sb")
        bks_sb = st([16, 1], f32, "bks_sb")
        c0k_sb = st([16, 1], f32, "c0k_sb")
        br_row = st([1, D], f32, "br_row")
        bout_sb = st([1, D], bf16, "bout_sb")
        boutB = st([P, D], f32, "boutB")

        eE = st([16, S], bf16, "eE")
        z_nat = st([P, NSB], f32, "z_nat")
        rz_nat = st([P, NSB], f32, "rz_nat")
        alpha = st([P, NSB * H], bf16, "alpha")
        beta = st([P, NSB * H], bf16, "beta")
        at_sb = st([16, D], bf16, "at_sb")
        gts = st([16, D], bf16, "gts")          # graw^T / Wfold^T staging
        a_nat = st([P, NCC * H], bf16, "a_nat")
        s_col = st([16, 1], bf16, "s_col")
        s_row = st([1, 16], bf16, "s_row")
        ext_tmp = st([P, NDB * H], f32, "ext_tmp")
        gq_sb = st([P, NDB], f32, "gq_sb")      # = SCALE * gq
        gkd_sb = st([P, NDB], f32, "gkd_sb")    # = gkd / SCALE
        gk_sb = st([P, NDB], f32, "gk_sb")

        # ---------- small DMAs on scalar (idle until the first exps) -------
        nc.scalar.dma_start(packW_sb[:], PackW[:, :])
        nc.scalar.dma_start(c0q_sb[:], c0q.unsqueeze(1))
        nc.scalar.dma_start(bks_sb[:], bks_s.unsqueeze(1))
        nc.scalar.dma_start(packR_sb[:], PackR[:, :])
        nc.scalar.dma_start(packA_sb[:], PackA[:, :])
        nc.scalar.dma_start(br_row[:], br_f.unsqueeze(0))

        # ---------- bulk DMA: stripe BOTH queues, global first-need order ---
        # Each queue's steady state is ~2.1us fixed per transfer (semaphore
        # grant + reissue) plus wire time, so 512KB transfers crawl at
        # ~155GB/s while the burst rate is ~430GB/s.  1MB transfers amortize
        # the overhead to ~230GB/s/queue; two queues then reach the HBM cap.
        def xt_pair(cc, eng):
            eng.dma_start(xt[:, cc * S:(cc + 2) * S],
                          XT[:, cc * S:(cc + 2) * S])

        def w_half(dst, src_, half, eng):
            lo = half * (NCC // 2)
            eng.dma_start(dst[:, lo * D:(lo + NCC // 2) * D],
                          src_[:, lo * D:(lo + NCC // 2) * D])

        def xn_quad(si, eng):
            eng.dma_start(xnat[:, si * D:(si + 4) * D],
                          X[:, si * D:(si + 4) * D])

        xt_pair(0, nc.sync)
        xt_pair(4, nc.gpsimd)
        xt_pair(2, nc.sync)
        xt_pair(6, nc.gpsimd)
        xn_quad(0, nc.sync)
        xn_quad(4, nc.gpsimd)
        xn_quad(8, nc.sync)
        xn_quad(12, nc.gpsimd)
        for src_, dst in ((Wqv, wqv_all), (WkT, wkT_all), (Wk, wk_all),
                          (Wr, wr_all), (WqvT, wqvT_all)):
            w_half(dst, src_, 0, nc.sync)
            w_half(dst, src_, 1, nc.gpsimd)

        # ---------- tiny constants on vector ----------
        ones_col = st([P, 1], bf16, "ones_col")
        nc.vector.memset(ones_col[:], 1.0)
        ones_row = st([1, P], bf16, "ones_row")
        nc.vector.memset(ones_row[:], 1.0)

        # ---------- PE warm-up: junk matmuls on packW while DMA streams ----
        def warm(n):
            for _ in range(n):
                wt = pt_([P, SH], "big", 2)
                nc.tensor.matmul(wt[:, :P], packW_sb[:, :P], packW_sb[:, :P],
                                 start=True, stop=True)

        # ---------- fused softmax -> weights -> A -> graw -> extract -------
        def softmax_ws(w16, bias_ap, wout, wall, b_row_ap, mask3, g_out,
                       sh_outer):
            lgs = [pt_([16, SH], "lg", 4) for _ in range(NSH)]
            if sh_outer:
                for sh in range(NSH):
                    for cb in range(NCC):
                        nc.tensor.matmul(
                            lgs[sh][:], w16[:, cb * H:(cb + 1) * H],
                            xt[:, cb * S + sh * SH: cb * S + sh * SH + SH],
                            start=(cb == 0), stop=(cb == NCC - 1))
                    nc.scalar.activation(eE[:, sh * SH:(sh + 1) * SH],
                                         lgs[sh][:], AF.Exp,
                                         bias=bias_ap, scale=1.0)
            else:
                # cc-outer: consume xt chunks as the DMA lands them
                for cb in range(NCC):
                    for sh in range(NSH):
                        nc.tensor.matmul(
                            lgs[sh][:], w16[:, cb * H:(cb + 1) * H],
                            xt[:, cb * S + sh * SH: cb * S + sh * SH + SH],
                            start=(cb == 0), stop=(cb == NCC - 1))
                    if cb >= 2:
                        warm(1)
                for sh in range(NSH):
                    nc.scalar.activation(eE[:, sh * SH:(sh + 1) * SH],
                                         lgs[sh][:], AF.Exp,
                                         bias=bias_ap, scale=1.0)

            a0 = pt_([16, SH], "lg", 4)
            a1 = pt_([16, SH], "lg", 4)
            sps_t = pt_([P, SH], "big", 2)
            sps = sps_t[:16, :1]

            def a_mms(sb):
                lhs = wout[:, sb * H:(sb + 1) * H]
                nc.tensor.matmul(a0[:], lhs,
                                 xnat[:, sb * D: sb * D + SH],
                                 start=(sb == 0), stop=(sb == NSB - 1))
                nc.tensor.matmul(a1[:], lhs,
                                 xnat[:, sb * D + SH: sb * D + 2 * SH],
                                 start=(sb == 0), stop=(sb == NSB - 1))

            ngrp = NSB // NG
            for g in range(ngrp):
                eT_ps = pt_([P, 4 * P], "sc", 2, dt_=bf16)
                for t in range(NG):
                    sb = NG * g + t
                    nc.tensor.transpose(eT_ps[:, t * H:(t + 1) * H],
                                        eE[:, sb * P:(sb + 1) * P], id16)
                sl = slice(NG * g * H, (NG * g + NG) * H)
                gz = slice(NG * g, NG * g + NG)
                nc.vector.reduce_sum(
                    z_nat[:, gz].unsqueeze(2),
                    eT_ps[:, :NG * H].rearrange("p (sb h) -> p sb h", sb=NG),
                    axis=AXX)
                nc.vector.reciprocal(rz_nat[:, gz], z_nat[:, gz])
                nc.vector.tensor_tensor(
                    wout[:, sl].rearrange("p (sb h) -> p sb h", sb=NG),
                    eT_ps[:, :NG * H].rearrange("p (sb h) -> p sb h", sb=NG),
                    rz_nat[:, gz].unsqueeze(2).broadcast_to([P, NG, H]),
                    ALU.mult)
                # previous group's A matmuls fill PE while this group's
                # exp/alpha chain completes
                if g > 0:
                    for sb in range(NG * (g - 1), NG * g):
                        a_mms(sb)
            for sb in range(NG * (ngrp - 1), NSB):
                a_mms(sb)
            for sb in range(NSB):
                nc.tensor.matmul(sps, wout[:, sb * H:(sb + 1) * H],
                                 ones_col[:],
                                 start=(sb == 0), stop=(sb == NSB - 1))

            # ---- A^T -> natural ----
            nc.scalar.copy(at_sb[:, :SH], a0[:])
            nc.vector.tensor_copy(at_sb[:, SH:], a1[:])
            nc.vector.tensor_copy(s_col[:], sps)
            trA = pt_([P, 4 * P], "sc", 2, dt_=bf16)
            for cc in range(NCC):
                nc.tensor.transpose(trA[:, cc * H:(cc + 1) * H],
                                    at_sb[:, cc * P:(cc + 1) * P], id16)
            nc.tensor.transpose(trA[:1, NSB * H:NSB * H + 16], s_col[:], id16)
            nc.vector.tensor_copy(a_nat[:], trA[:, :NCC * H])
            nc.vector.tensor_copy(s_row[:], trA[:1, NSB * H:NSB * H + 16])

            # ---- graw^T = A^T-stationary x W-moving (+ s b^T), then back ---
            gA = pt_([16, SH], "lg", 4)
            gB = pt_([16, SH], "lg", 4)
            for dd in range(NCC):
                nc.tensor.matmul(gA[:], a_nat[:, dd * H:(dd + 1) * H],
                                 wall[:, dd * D: dd * D + SH],
                                 start=(dd == 0), stop=False)
                nc.tensor.matmul(gB[:], a_nat[:, dd * H:(dd + 1) * H],
                                 wall[:, dd * D + SH: (dd + 1) * D],
                                 start=(dd == 0), stop=False)
            nc.tensor.matmul(gA[:], s_row[:1, :], b_row_ap[:1, :SH],
                             start=False, stop=True)
            nc.tensor.matmul(gB[:], s_row[:1, :], b_row_ap[:1, SH:],
                             start=False, stop=True)
            nc.scalar.copy(gts[:, :SH], gA[:])
            nc.vector.tensor_copy(gts[:, SH:], gB[:])
            grT = pt_([P, 4 * P], "sc", 2, dt_=bf16)
            for jb in range(NDB):
                nc.tensor.transpose(grT[:, jb * H:(jb + 1) * H],
                                    gts[:, jb * P:(jb + 1) * P], id16)
            nc.vector.tensor_tensor(ext_tmp[:], grT[:, :NDB * H], mask3[:],
                                    ALU.mult)
            nc.vector.reduce_sum(
                g_out[:].unsqueeze(2),
                ext_tmp[:].rearrange("p (j h) -> p j h", j=NDB),
                axis=AXX)

        # ---------- phase 1: alphas -> gq ----------
        warm(4)
        softmax_ws(wsm_sb, c0q_sb[:, :1], alpha, wqv_all, bqv_row, mask3S,
                   gq_sb, sh_outer=False)

        # ---------- phase 2: t = sc*diag(gq)*Wk_s ; Wfold ; betas -> gkd ----
        nc.vector.tensor_tensor(
            t_sb[:].rearrange("p (j h) -> p j h", j=NDB),
            wks_sb[:].rearrange("p (j h) -> p j h", j=NDB),
            gq_sb[:].unsqueeze(2).broadcast_to([P, NDB, H]),
            ALU.mult)
        c0k_t = pt_([P, SH], "big", 2)
        c0k_ps = c0k_t[:16, :1]
        for j in range(NDB):
            nc.tensor.matmul(c0k_ps, t_sb[:, j * H:(j + 1) * H],
                             bkpp[:, j:j + 1],
                             start=(j == 0), stop=(j == NDB - 1))
        nc.vector.tensor_tensor(c0k_sb[:], c0k_ps, bks_sb[:], ALU.add)
        # Wfold^T = t-stationary x WkT-moving (dd-progressive), then back
        wfA = pt_([16, SH], "lg", 4)
        wfB = pt_([16, SH], "lg", 4)
        for dd in range(NCC):
            nc.tensor.matmul(wfA[:], t_sb[:, dd * H:(dd + 1) * H],
                             wkT_all[:, dd * D: dd * D + SH],
                             start=(dd == 0), stop=(dd == NCC - 1))
            nc.tensor.matmul(wfB[:], t_sb[:, dd * H:(dd + 1) * H],
                             wkT_all[:, dd * D + SH: (dd + 1) * D],
                             start=(dd == 0), stop=(dd == NCC - 1))
        nc.scalar.copy(gts[:, :SH], wfA[:])
        nc.vector.tensor_copy(gts[:, SH:], wfB[:])
        wf_t = pt_([P, 4 * P], "sc", 2, dt_=bf16)
        for jb in range(NDB):
            nc.tensor.transpose(wf_t[:, jb * H:(jb + 1) * H],
                                gts[:, jb * P:(jb + 1) * P], id16)
        nc.vector.tensor_copy(wfold[:], wf_t[:, :NDB * H])
        softmax_ws(wfold, c0k_sb[:, :1], beta, wk_all, bk_row, mask3K,
                   gkd_sb, sh_outer=True)

        # ---------- phase 3: gk ; M = I + diag(gk) Wr ----------
        nc.vector.tensor_mul(gk_sb[:], gq_sb[:], gkd_sb[:])
        for cc in range(NCC):
            sl = slice(cc * D, (cc + 1) * D)
            if cc % 2 == 0:
                nc.vector.tensor_scalar(wr_all[:, sl], wr_all[:, sl],
                                        gk_sb[:, cc:cc + 1], None, ALU.mult)
            else:
                nc.scalar.activation(wr_all[:, sl], wr_all[:, sl], AF.Copy,
                                     bias=0.0, scale=gk_sb[:, cc:cc + 1])
            nc.vector.tensor_add(
                wr_all[:, cc * D + cc * P: cc * D + (cc + 1) * P],
                wr_all[:, cc * D + cc * P: cc * D + (cc + 1) * P], eye_bf)
        # ---------- phase 4: Wbig = W_qv @ M  (chases the per-chunk scales) -
        for cb in range(NCC):
            for eh in range(NDH):
                ps = pt_([P, SH], "big", 2)
                for dd in range(NCC):
                    nc.tensor.matmul(
                        ps[:], wqvT_all[:, dd * D + cb * P: dd * D + cb * P + P],
                        wr_all[:, dd * D + eh * SH: dd * D + (eh + 1) * SH],
                        start=(dd == 0), stop=(dd == NCC - 1))
                nc.scalar.copy(wbig[:, cb * D + eh * SH: cb * D + (eh + 1) * SH],
                               ps[:])

        # b_out = b_qv @ M + b_r
        for eh in range(NDH):
            bo = pt_([16, SH], "lg", 4)
            for j in range(NDB):
                nc.tensor.matmul(bo[:1, :], bqvpp[:, j:j + 1],
                                 wr_all[:, j * D + eh * SH: j * D + (eh + 1) * SH],
                                 start=(j == 0), stop=(j == NDB - 1))
            nc.vector.tensor_tensor(bout_sb[:1, eh * SH:(eh + 1) * SH],
                                    bo[:1, :], br_row[:1, eh * SH:(eh + 1) * SH],
                                    ALU.add)
        for eh in range(NDH):
            bb = pt_([P, SH], "big", 2)
            nc.tensor.matmul(bb[:], ones_row[:1, :],
                             bout_sb[:1, eh * SH:(eh + 1) * SH],
                             start=True, stop=True)
            nc.vector.tensor_copy(boutB[:, eh * SH:(eh + 1) * SH], bb[:])

        # ---------- phase 5: out = X @ Wbig + b_out ----------
        for sb in range(NSB):
            for eh in range(NDH):
                ps = pt_([P, SH], "big", 2)
                for cc in range(NCC):
                    nc.tensor.matmul(
                        ps[:], xt[:, cc * S + sb * P: cc * S + sb * P + P],
                        wbig[:, cc * D + eh * SH: cc * D + (eh + 1) * SH],
                        start=(cc == 0), stop=(cc == NCC - 1))
                if sb == NSB - 1:
                    hw = SH // 2
                    for q in range(2):
                        obq = st([P, hw], bf16, "obq", bufs=4)
                        nc.vector.tensor_tensor(
                            obq[:], ps[:, q * hw:(q + 1) * hw],
                            boutB[:, eh * SH + q * hw: eh * SH + (q + 1) * hw],
                            ALU.add)
                        nc.sync.dma_start(
                            OUT[sb * P:(sb + 1) * P,
                                eh * SH + q * hw: eh * SH + (q + 1) * hw],
                            obq[:])
                else:
                    ob = st([P, SH], bf16, "ob", bufs=4)
                    nc.vector.tensor_tensor(
                        ob[:], ps[:], boutB[:, eh * SH:(eh + 1) * SH], ALU.add)
                    eng = nc.gpsimd if sb < 12 and (sb * NDH + eh) % 2 == 0 \
                        else nc.sync
                    eng.dma_start(
                        OUT[sb * P:(sb + 1) * P, eh * SH:(eh + 1) * SH],
                        ob[:])

    nc.compile()
    return nc


def _get_nc():
    if "nc" not in _CACHE:
        _CACHE["nc"] = _build()
    return _CACHE["nc"]


def _prep_inputs(inputs):
    import ml_dtypes
    bf = ml_dtypes.bfloat16

    def f(k):
        return np.ascontiguousarray(np.asarray(inputs[k], dtype=np.float32))

    def c(a):
        return np.ascontiguousarray(np.asarray(a, dtype=np.float32).astype(bf))

    W_qv, W_k, W_r = f("W_qv"), f("W_k"), f("W_r")
    Wq_s, Wk_s = f("Wq_s"), f("Wk_s")
    b_qv, b_k, b_r = f("b_qv"), f("b_k"), f("b_r")
    bq_s, bk_s = f("bq_s"), f("bk_s")

    def perm(w):
        # [C*128, N] -> [128, C*N]: row-block cc goes to columns cc*N
        cb = w.shape[0] // P
        return w.reshape(cb, P, w.shape[1]).transpose(1, 0, 2).reshape(P, -1)

    # mask3[p, j*H + h] = v iff h == 2j + p//64
    pj = np.arange(P)[:, None] // 64 + 2 * np.arange(NDB)[None, :]  # [P, j]
    m3 = (pj[:, :, None] == np.arange(H)[None, None, :])            # [P, j, H]
    mask3S = (m3 * SCALE).reshape(P, NDB * H)
    mask3K = (m3 / SCALE).reshape(P, NDB * H)

    packW = np.concatenate([perm(SCALE * (W_qv @ Wq_s)), np.eye(P)], axis=1)
    packA = np.concatenate(
        [perm(Wk_s), b_qv.reshape(NDB, P).T, b_k.reshape(NDB, P).T,
         mask3S, mask3K], axis=1)
    packR = np.concatenate([b_qv, b_k]).reshape(1, 2 * D)

    common = {
        "Wqvb": c(perm(W_qv)), "WqvTb": c(perm(W_qv.T)),
        "Wkb": c(perm(W_k)), "WkTb": c(perm(W_k.T)),
        "Wrb": c(perm(W_r)),
        "packW": c(packW),
        "packA": c(packA),
        "packR": c(packR),
        "c0qf": np.ascontiguousarray(SCALE * (b_qv @ Wq_s + bq_s)),
        "bks_sf": np.ascontiguousarray(SCALE * bk_s),
        "br_f32": b_r,
    }
    in_maps = []
    for b in range(NCORES):
        m = dict(common)
        xb = np.asarray(inputs["X"][b], dtype=np.float32)
        m["Xb"] = c(perm(xb))
        m["XTb"] = c(perm(xb.T))
        in_maps.append(m)
    return in_maps


def run(inputs, trace=False):
    from concourse.bass_utils import run_bass_kernel_spmd

    nc = _get_nc()
    in_maps = _prep_inputs(inputs)
    res = run_bass_kernel_spmd(nc, in_maps, core_ids=list(range(NCORES)),
                               trace=trace)
    _CACHE["last_results"] = res
    out = np.stack([np.asarray(res.results[b]["out"], dtype=np.float32)
                    for b in range(NCORES)], axis=0)
    return out


def kernel(**inputs):
    trace = os.environ.get("KTRACE", "0") == "1"
    return run(inputs, trace=trace)
